# revision 1
# baseline (speedup 1.0000x reference)
"""GATv2 (3-layer, H=1, D=128) on 8 Trainium2 NeuronCores via Bass/Tile.

Self-contained: host preprocessing (bin packing, edge tiling, int16 gather
indices), bass program builder, and the `kernel(**inputs)` entry point.

Strategy (dst-node sharding, uniform SPMD program):
  - Permute nodes into bins of 128 dst rows, 2-D balanced by (low,high)
    in-degree so every bin has exactly TL low + TH high edge tiles of 128.
  - Per layer: each core computes xl/xr for its shard (bf16), AllGather of
    xl -> xl_full in every core's HBM.
  - Per bin: dma_gather source features u[e,d] (bf16, 256B rows); per edge
    tile: PE transpose u -> wT psum, += xr^T-onehot matmul (U streamed from
    HBM); ACT LeakyRelu(+bl+br bias); score = lreluT^T @ att (PE);
    exp (ACT); denom/agg via one-hot matmuls accumulated in PSUM; epilogue
    recip + bias + relu (+ log_softmax on the last layer).
"""
import numpy as np
import ml_dtypes

import concourse.bacc as bacc
import concourse.mybir as mybir
import concourse.tile as tile
from concourse.bass_utils import run_bass_kernel_spmd

F32 = mybir.dt.float32
BF16 = mybir.dt.bfloat16
I16 = mybir.dt.int16
SLOPE = 0.2
D = 128
LOWCAP = 32768
DUMMY_DCOL = 200.0  # never matches iota 0..127 -> zero one-hot column


# ----------------------------------------------------------------------------
# Host preprocessing
# ----------------------------------------------------------------------------

def _pack_bins(n_bins, node_ids, w_low, w_high, cap=128):
    order = np.argsort(-(w_low + w_high), kind="stable")
    bins = [[] for _ in range(n_bins)]
    sums = np.zeros((n_bins, 2))
    counts = np.zeros(n_bins, dtype=np.int64)
    tgt_l = max(w_low.sum() / n_bins, 1.0)
    tgt_h = max(w_high.sum() / n_bins, 1.0)
    for idx in order:
        load = (sums[:, 0] + w_low[idx]) / tgt_l + (sums[:, 1] + w_high[idx]) / tgt_h
        load = np.where(counts >= cap, np.inf, load)
        b = int(np.argmin(load))
        bins[b].append(node_ids[idx])
        sums[b, 0] += w_low[idx]
        sums[b, 1] += w_high[idx]
        counts[b] += 1
    return bins, sums


def _chunks(n, cap=12):
    out = []
    while n > 0:
        c = min(n, cap)
        out.append(c)
        n -= c
    return out


class Plan:
    pass


def host_prep(x, edge_index, n_cores):
    """Returns Plan with per-core input tensors + structural constants."""
    N = x.shape[0]
    E = edge_index.shape[1]
    src = np.concatenate([np.asarray(edge_index[0], dtype=np.int64),
                          np.arange(N, dtype=np.int64)])
    dst = np.concatenate([np.asarray(edge_index[1], dtype=np.int64),
                          np.arange(N, dtype=np.int64)])

    NB = int(np.ceil(N / 128.0))
    NB = int(np.ceil(NB / n_cores)) * n_cores
    NPAD = NB * 128
    assert NPAD <= 65536
    lowcap = min(LOWCAP, NPAD)  # small configs: everything "low"
    hibase = NPAD - lowcap

    deg = np.bincount(dst, minlength=N).astype(np.float64)
    if NPAD > lowcap:
        # pass 1: balance total degree; defines the low/high node split
        bins1, _ = _pack_bins(NB, np.arange(N), deg, np.zeros(N))
        n_low_bins = lowcap // 128
        low_nodes = np.concatenate(
            [np.asarray(bins1[i], dtype=np.int64) for i in range(n_low_bins)]
        ) if n_low_bins else np.zeros(0, dtype=np.int64)
        is_low = np.zeros(N, dtype=bool)
        is_low[low_nodes] = True
        deg_low = np.bincount(dst[is_low[src]], minlength=N).astype(np.float64)
        deg_high = deg - deg_low
        ln = np.flatnonzero(is_low)
        hn = np.flatnonzero(~is_low)
        bins_l, _ = _pack_bins(n_low_bins, ln, deg_low[ln], deg_high[ln])
        bins_h, _ = _pack_bins(NB - n_low_bins, hn, deg_low[hn], deg_high[hn])
        bins = bins_l + bins_h
    else:
        bins, _ = _pack_bins(NB, np.arange(N), deg, np.zeros(N))
        is_low = np.ones(N, dtype=bool)

    # permutation: pos[old] = new index; bins padded with virtual rows
    pos = np.full(N, -1, dtype=np.int64)
    bin_nodes = np.full((NB, 128), -1, dtype=np.int64)
    for b, nodes in enumerate(bins):
        nodes = np.asarray(nodes, dtype=np.int64)
        bin_nodes[b, :len(nodes)] = nodes
        pos[nodes] = b * 128 + np.arange(len(nodes))

    # per-edge: bin, dst_local, low/high, gather idx
    e_pos_dst = pos[dst]
    e_bin = e_pos_dst // 128
    e_row = e_pos_dst % 128
    e_pos_src = pos[src]
    e_is_low = e_pos_src < lowcap

    # group edges per (bin, range)
    # sort by (bin, is_low desc) so lows first
    order = np.lexsort((~e_is_low, e_bin))
    sb, srow, sidx, slow = e_bin[order], e_row[order], e_pos_src[order], e_is_low[order]

    nl = np.zeros(NB, dtype=np.int64)
    nh = np.zeros(NB, dtype=np.int64)
    np.add.at(nl, e_bin[e_is_low], 1)
    np.add.at(nh, e_bin[~e_is_low], 1)
    # virtual rows get a self-edge in the low group
    nvirt = (bin_nodes < 0).sum(axis=1)
    TL = int(np.ceil((nl + nvirt).max() / 128.0))
    TH = int(np.ceil(nh.max() / 128.0)) if NPAD > lowcap else 0
    TPB = TL + TH
    low_calls = _chunks(TL)
    high_calls = _chunks(TH)
    calls_per_bin = [(c, 0) for c in low_calls] + [(c, 1) for c in high_calls]

    # slot arrays per bin: idx (int16), dcol (float), U one-hot
    slot_idx = np.zeros((NB, TPB * 128), dtype=np.int64)
    slot_dcol = np.full((NB, TPB * 128), DUMMY_DCOL, dtype=np.float64)
    # fill real edges
    # low edges occupy slots [0, nl), high edges [TL*128, TL*128+nh)
    starts = np.searchsorted(sb, np.arange(NB))
    ends = np.searchsorted(sb, np.arange(NB), side="right")
    for b in range(NB):
        s, e = starts[b], ends[b]
        lo_cnt = int(np.count_nonzero(slow[s:e]))
        hi_cnt = (e - s) - lo_cnt
        assert lo_cnt <= TL * 128 - nvirt[b] and hi_cnt <= TH * 128
        slot_idx[b, :lo_cnt] = sidx[s:s + lo_cnt]
        slot_dcol[b, :lo_cnt] = srow[s:s + lo_cnt]
        # virtual self-edges right after real low edges
        nv = int(nvirt[b])
        if nv:
            vrows = np.flatnonzero(bin_nodes[b] < 0)
            slot_idx[b, lo_cnt:lo_cnt + nv] = 0
            slot_dcol[b, lo_cnt:lo_cnt + nv] = vrows
        if hi_cnt:
            slot_idx[b, TL * 128:TL * 128 + hi_cnt] = sidx[s + lo_cnt:e] - hibase
            slot_dcol[b, TL * 128:TL * 128 + hi_cnt] = srow[s + lo_cnt:e]
    assert slot_idx.min() >= 0 and slot_idx.max() < 32768

    # wrapped int16 gather indices per call, replicated to 8 groups of 16
    def wrap(idx_flat):
        ni = len(idx_flat)
        a = idx_flat.astype(np.int16).reshape(ni // 16, 16).T  # [16, ni/16]
        w = np.tile(a, (8, 1))  # [128, ni/16]
        return w

    core_gidx = []
    core_dcol = []
    core_U = []
    core_x = []
    NBPC = NB // n_cores
    for c in range(n_cores):
        gx = []
        for b in range(c * NBPC, (c + 1) * NBPC):
            off = 0
            for ntile, rng in calls_per_bin:
                lo = (0 if rng == 0 else TL * 128) + (off if rng == 0 else off - TL * 128)
                gx.append(wrap(slot_idx[b, off:off + ntile * 128]))
                off += ntile * 128
            assert off == TPB * 128
        core_gidx.append(np.concatenate(gx, axis=1))
        dc = slot_dcol[c * NBPC:(c + 1) * NBPC].reshape(NBPC * TPB, 128)
        # dcol layout [128 partitions(e), NBPC*TPB]
        core_dcol.append(
            np.ascontiguousarray(dc.T).astype(ml_dtypes.bfloat16))
        # U one-hot [128 i, tiles*128 e]
        dloc = slot_dcol[c * NBPC:(c + 1) * NBPC].reshape(-1)  # [NBPC*TPB*128]
        U = (dloc[None, :] == np.arange(128)[:, None]).astype(ml_dtypes.bfloat16)
        core_U.append(np.ascontiguousarray(U))
        # x shard in permuted order (virtual rows zero)
        xs = np.zeros((NBPC * 128, D), dtype=np.float32)
        nodes = bin_nodes[c * NBPC:(c + 1) * NBPC].reshape(-1)
        valid = nodes >= 0
        xs[valid] = np.asarray(x)[nodes[valid]]
        core_x.append(xs)

    p = Plan()
    p.N, p.E, p.NB, p.NPAD, p.NBPC = N, E, NB, NPAD, NBPC
    p.TL, p.TH, p.TPB = TL, TH, TPB
    p.calls_per_bin = calls_per_bin
    p.lowcap, p.hibase = lowcap, hibase
    p.n_cores = n_cores
    p.core_gidx, p.core_dcol, p.core_U, p.core_x = (
        core_gidx, core_dcol, core_U, core_x)
    p.bin_nodes = bin_nodes
    p.pos = pos
    p.idx_cols = core_gidx[0].shape[1]
    return p


def host_weights(Wl, bl, Wr, br, att, bias, L):
    """Shared (replicated) weight tensors."""
    w = {}
    w["wl"] = np.ascontiguousarray(
        np.concatenate([np.asarray(Wl[l], np.float32) for l in range(L)], axis=1)
    ).astype(ml_dtypes.bfloat16)  # [128, L*128]
    w["wr"] = np.ascontiguousarray(
        np.concatenate([np.asarray(Wr[l], np.float32) for l in range(L)], axis=1)
    ).astype(ml_dtypes.bfloat16)
    w["att"] = np.ascontiguousarray(
        np.stack([np.asarray(att[l, 0], np.float32) for l in range(L)], axis=1)
    ).astype(ml_dtypes.bfloat16)  # [128, L]
    w["att2"] = np.ascontiguousarray(
        np.stack([np.asarray(-0.2 * att[l, 0], np.float32) for l in range(L)],
                 axis=1)).astype(ml_dtypes.bfloat16)  # [128, L]
    # lrelu bias = bl + br per layer, [128, L] f32
    w["lrb"] = np.ascontiguousarray(
        np.stack([np.asarray(bl[l] + br[l], np.float32) for l in range(L)], axis=1))
    w["lrbn"] = np.ascontiguousarray(-w["lrb"])
    # epilogue bias = bl + bias per layer, replicated [128, L*128] f32
    be = np.concatenate(
        [np.tile((np.asarray(bl[l] + bias[l], np.float32))[None, :], (128, 1))
         for l in range(L)], axis=1)
    w["be"] = np.ascontiguousarray(be)
    ident = np.eye(128, dtype=np.float32)
    w["idf"] = ident
    w["idb"] = ident.astype(ml_dtypes.bfloat16)
    w["iota"] = np.tile(np.arange(128, dtype=np.float32)[None, :], (128, 4)
                        ).astype(ml_dtypes.bfloat16)  # [128, 4*128]
    return w


# ----------------------------------------------------------------------------
# Device program
# ----------------------------------------------------------------------------

def build_nc(p, L=3):
    NBPC, TL, TH, TPB = p.NBPC, p.TL, p.TH, p.TPB
    NSH = NBPC * 128          # shard rows
    n_cores = p.n_cores
    NPAD = p.NPAD

    nc = bacc.Bacc("TRN2", target_bir_lowering=False, debug=False,
                   num_devices=n_cores, num_swdge_queues=4)

    # --- I/O ---
    x_in = nc.dram_tensor("x_shard", [NSH, D], F32, kind="ExternalInput")
    gidx_in = nc.dram_tensor("gidx", [128, p.idx_cols], I16, kind="ExternalInput")
    dcol_in = nc.dram_tensor("dcol", [128, NBPC * TPB], BF16, kind="ExternalInput")
    U_in = nc.dram_tensor("umat", [128, NBPC * TPB * 128], BF16,
                          kind="ExternalInput")
    wl_in = nc.dram_tensor("wl", [128, L * 128], BF16, kind="ExternalInput")
    wr_in = nc.dram_tensor("wr", [128, L * 128], BF16, kind="ExternalInput")
    att_in = nc.dram_tensor("att", [128, L], BF16, kind="ExternalInput")
    att2_in = nc.dram_tensor("att2", [128, L], BF16, kind="ExternalInput")
    lrb_in = nc.dram_tensor("lrb", [128, L], F32, kind="ExternalInput")
    lrbn_in = nc.dram_tensor("lrbn", [128, L], F32, kind="ExternalInput")
    be_in = nc.dram_tensor("be", [128, L * 128], F32, kind="ExternalInput")
    idf_in = nc.dram_tensor("idf", [128, 128], F32, kind="ExternalInput")
    idb_in = nc.dram_tensor("idb", [128, 128], BF16, kind="ExternalInput")
    iota_in = nc.dram_tensor("iota", [128, 4 * 128], BF16, kind="ExternalInput")
    out_t = nc.dram_tensor("out", [NSH, D], F32, kind="ExternalOutput")

    with tile.TileContext(nc) as tc:
        with (
            tc.tile_pool(name="const", bufs=1) as constp,
            tc.tile_pool(name="sb", bufs=2) as sb,
            tc.tile_pool(name="sb3", bufs=6) as sb3,
            tc.tile_pool(name="psum", bufs=2, space="PSUM") as ps,
            tc.tile_pool(name="dram", bufs=1, space="DRAM") as dram,
        ):
            # ---- constants / resident tensors ----
            gidx = constp.tile([128, p.idx_cols], I16)
            nc.sync.dma_start(gidx[:], gidx_in[:])
            dcol = constp.tile([128, NBPC * TPB], BF16)
            nc.sync.dma_start(dcol[:], dcol_in[:])
            wl = constp.tile([128, L * 128], BF16)
            nc.sync.dma_start(wl[:], wl_in[:])
            wr = constp.tile([128, L * 128], BF16)
            nc.sync.dma_start(wr[:], wr_in[:])
            att = constp.tile([128, L], BF16)
            nc.sync.dma_start(att[:], att_in[:])
            att2 = constp.tile([128, L], BF16)
            nc.sync.dma_start(att2[:], att2_in[:])
            lrb = constp.tile([128, L], F32)
            nc.sync.dma_start(lrb[:], lrb_in[:])
            lrbn = constp.tile([128, L], F32)
            nc.sync.dma_start(lrbn[:], lrbn_in[:])
            be = constp.tile([128, L * 128], F32)
            nc.sync.dma_start(be[:], be_in[:])
            idf = constp.tile([128, 128], F32)
            nc.sync.dma_start(idf[:], idf_in[:])
            idb = constp.tile([128, 128], BF16)
            nc.sync.dma_start(idb[:], idb_in[:])
            iota = constp.tile([128, 4 * 128], BF16)
            nc.sync.dma_start(iota[:], iota_in[:])
            # xr kept SBUF-resident per layer
            xr_res = constp.tile([128, NBPC * 128], BF16)

            # ---- DRAM scratch ----
            h_cur = dram.tile([NSH, D], F32)       # current layer input
            xl_shard = dram.tile([NSH, D], BF16)
            xl_full = dram.tile([NPAD, D], BF16)
            nc.sync.dma_start(h_cur[:], x_in[:])

            for l in range(L):
                # ======== Phase A: xl/xr shard + AllGather ========
                for t in range(NBPC):
                    h_t = sb.tile([128, 128], F32, tag="ht")
                    nc.sync.dma_start(h_t[:], h_cur[t * 128:(t + 1) * 128, :])
                    hT_ps = ps.tile([128, 128], F32, tag="wT")
                    nc.tensor.matmul(hT_ps[:], lhsT=h_t[:], rhs=idf[:],
                                     start=True, stop=True)
                    hT = sb.tile([128, 128], BF16, tag="hT")
                    nc.scalar.copy(hT[:], hT_ps[:])
                    xl_ps = ps.tile([128, 128], F32, tag="ag")
                    nc.tensor.matmul(xl_ps[:], lhsT=hT[:],
                                     rhs=wl[:, l * 128:(l + 1) * 128],
                                     start=True, stop=True)
                    xl_t = sb.tile([128, 128], BF16, tag="xlt")
                    nc.scalar.copy(xl_t[:], xl_ps[:])
                    nc.sync.dma_start(xl_shard[t * 128:(t + 1) * 128, :], xl_t[:])
                    xr_ps = ps.tile([128, 128], F32, tag="sc")
                    nc.tensor.matmul(xr_ps[:], lhsT=hT[:],
                                     rhs=wr[:, l * 128:(l + 1) * 128],
                                     start=True, stop=True)
                    nc.scalar.copy(xr_res[:, t * 128:(t + 1) * 128], xr_ps[:])

                nc.gpsimd.collective_compute(
                    "AllGather",
                    mybir.AluOpType.bypass,
                    replica_groups=[list(range(n_cores))],
                    ins=[xl_shard.opt()],
                    outs=[xl_full.opt()],
                )

                # ======== Phase B: per-bin edge processing ========
                idx_off = 0
                for b in range(NBPC):
                    u_bin = sb3.tile([128, TPB, 128], BF16, tag="u")
                    # gather calls
                    tile_off = 0
                    for ci, (ntile, rng) in enumerate(p.calls_per_bin):
                        ni = ntile * 128
                        base = 0 if rng == 0 else p.hibase
                        src_view = xl_full[base:base + p.lowcap, :]
                        nc.gpsimd.dma_gather(
                            out_ap=u_bin[:, tile_off:tile_off + ntile, :],
                            in_ap=src_view,
                            idxs_ap=gidx[:, idx_off:idx_off + ni // 16],
                            num_idxs=ni,
                            num_idxs_reg=ni,
                            elem_size=D,
                            queue_num=(b * len(p.calls_per_bin) + ci) % 4,
                            single_packet=False,
                        )
                        idx_off += ni // 16
                        tile_off += ntile
                    # U for this bin
                    U_bin = sb3.tile([128, TPB * 128], BF16, tag="U")
                    nc.sync.dma_start(
                        U_bin[:],
                        U_in[:, (b * TPB) * 128:((b + 1) * TPB) * 128])

                    xr_tile = xr_res[:, b * 128:(b + 1) * 128]
                    denom_ps = ps.tile([128, 1], F32, tag="dn")
                    agg_ps = ps.tile([128, 128], F32, tag="ag")

                    # batches of up to 4 tiles
                    bk_sizes = _chunks(TPB, 4)
                    t0 = 0
                    for bk in bk_sizes:
                        wT_ps = ps.tile([128, 4 * 128], F32, tag="wT")
                        for j in range(bk):
                            t = t0 + j
                            nc.tensor.matmul(
                                wT_ps[:, j * 128:(j + 1) * 128],
                                lhsT=u_bin[:, t, :], rhs=idb[:],
                                start=True, stop=False)
                            nc.tensor.matmul(
                                wT_ps[:, j * 128:(j + 1) * 128],
                                lhsT=xr_tile,
                                rhs=U_bin[:, t * 128:(t + 1) * 128],
                                start=False, stop=True)
                        lre = sb.tile([128, 4 * 128], BF16, tag="lre")
                        nc.scalar.activation(
                            lre[:, :bk * 128], wT_ps[:, :bk * 128],
                            mybir.ActivationFunctionType.Relu,
                            bias=lrb[:, l:l + 1], scale=1.0)
                        lrn = sb.tile([128, 4 * 128], BF16, tag="lrn")
                        nc.scalar.activation(
                            lrn[:, :bk * 128], wT_ps[:, :bk * 128],
                            mybir.ActivationFunctionType.Relu,
                            bias=lrbn[:, l:l + 1], scale=-1.0)
                        sc_ps = ps.tile([128, 4], F32, tag="sc")
                        for j in range(bk):
                            nc.tensor.matmul(
                                sc_ps[:, j:j + 1],
                                lhsT=lre[:, j * 128:(j + 1) * 128],
                                rhs=att[:, l:l + 1], start=True, stop=False)
                            nc.tensor.matmul(
                                sc_ps[:, j:j + 1],
                                lhsT=lrn[:, j * 128:(j + 1) * 128],
                                rhs=att2[:, l:l + 1], start=False, stop=True)
                        p8f = sb.tile([128, 4], F32, tag="p8f")
                        nc.scalar.activation(
                            p8f[:, :bk], sc_ps[:, :bk],
                            mybir.ActivationFunctionType.Exp)
                        p8 = sb.tile([128, 4], BF16, tag="p8")
                        nc.scalar.copy(p8[:, :bk], p8f[:, :bk])
                        A8 = sb.tile([128, 4, 128], BF16, tag="A8")
                        nc.vector.tensor_tensor(
                            out=A8[:, :bk, :],
                            in0=dcol[:, b * TPB + t0:b * TPB + t0 + bk]
                                .to_broadcast([128, bk, 128]),
                            in1=iota[:, :bk * 128].rearrange(
                                "p (b e) -> p b e", e=128),
                            op=mybir.AluOpType.is_equal)
                        for j in range(bk):
                            t = t0 + j
                            pu = sb.tile([128, 128], BF16, tag="pu")
                            nc.vector.tensor_tensor(
                                out=pu[:], in0=u_bin[:, t, :],
                                in1=p8f[:, j:j + 1].to_broadcast([128, 128]),
                                op=mybir.AluOpType.mult)
                            first = t == 0
                            last = t == TPB - 1
                            nc.tensor.matmul(
                                denom_ps[:], lhsT=A8[:, j, :],
                                rhs=p8[:, j:j + 1],
                                start=first, stop=last)
                            nc.tensor.matmul(
                                agg_ps[:], lhsT=A8[:, j, :], rhs=pu[:],
                                start=first, stop=last)
                        t0 += bk

                    # ---- epilogue ----
                    rec = sb.tile([128, 1], F32, tag="rec")
                    nc.vector.reciprocal(rec[:], denom_ps[:])
                    hp = sb.tile([128, 128], F32, tag="hp")
                    nc.vector.tensor_scalar_mul(hp[:], agg_ps[:], rec[:])
                    hb = sb.tile([128, 128], F32, tag="hb")
                    nc.vector.tensor_add(
                        hb[:], hp[:], be[:, l * 128:(l + 1) * 128])
                    if l < L - 1:
                        hn = sb.tile([128, 128], F32, tag="hn")
                        nc.scalar.activation(
                            hn[:], hb[:], mybir.ActivationFunctionType.Relu)
                        nc.sync.dma_start(
                            h_cur[b * 128:(b + 1) * 128, :], hn[:])
                    else:
                        z = sb.tile([128, 128], F32, tag="hn")
                        nc.scalar.activation(
                            z[:], hb[:], mybir.ActivationFunctionType.Relu)
                        m = sb.tile([128, 1], F32, tag="m")
                        nc.vector.tensor_reduce(
                            m[:], z[:], axis=mybir.AxisListType.X,
                            op=mybir.AluOpType.max)
                        zc = sb.tile([128, 128], F32, tag="zc")
                        nc.vector.tensor_scalar(
                            zc[:], z[:], scalar1=m[:], scalar2=None,
                            op0=mybir.AluOpType.subtract)
                        ez = sb.tile([128, 128], F32, tag="ez")
                        nc.scalar.activation(
                            ez[:], zc[:], mybir.ActivationFunctionType.Exp)
                        s = sb.tile([128, 1], F32, tag="s")
                        nc.vector.tensor_reduce(
                            s[:], ez[:], axis=mybir.AxisListType.X,
                            op=mybir.AluOpType.add)
                        ls = sb.tile([128, 1], F32, tag="ls")
                        nc.scalar.activation(
                            ls[:], s[:], mybir.ActivationFunctionType.Ln)
                        o = sb.tile([128, 128], F32, tag="o")
                        nc.vector.tensor_scalar(
                            o[:], zc[:], scalar1=ls[:], scalar2=None,
                            op0=mybir.AluOpType.subtract)
                        nc.sync.dma_start(
                            out_t[b * 128:(b + 1) * 128, :], o[:])

    nc.compile()
    return nc


# ----------------------------------------------------------------------------
# Entry point
# ----------------------------------------------------------------------------

def kernel(x, edge_index, Wl, bl, Wr, br, att, bias):
    n_cores = 8
    _sim = False
    x = np.asarray(x)
    edge_index = np.asarray(edge_index)
    L = np.asarray(Wl).shape[0]
    p = host_prep(x, edge_index, n_cores)
    w = host_weights(np.asarray(Wl), np.asarray(bl), np.asarray(Wr),
                     np.asarray(br), np.asarray(att), np.asarray(bias), L)
    nc = build_nc(p, L=L)

    in_maps = []
    for c in range(n_cores):
        m = {
            "x_shard": p.core_x[c],
            "gidx": p.core_gidx[c],
            "dcol": p.core_dcol[c],
            "umat": p.core_U[c],
            "wl": w["wl"], "wr": w["wr"], "att": w["att"],
            "att2": w["att2"], "lrb": w["lrb"], "lrbn": w["lrbn"],
            "be": w["be"],
            "idf": w["idf"], "idb": w["idb"], "iota": w["iota"],
        }
        in_maps.append(m)

    if _sim:
        from concourse.bass_interp import CoreSim, MultiCoreSim
        if n_cores == 1:
            sims = [CoreSim(nc)]
            for k, v in in_maps[0].items():
                sims[0].tensor(k)[:] = v
            sims[0].simulate()
            shard_outs = [sims[0].tensor("out").copy()]
        else:
            msim = MultiCoreSim(nc, num_cores=n_cores, trace=False)
            for c, core in sorted(msim.cores.items()):
                for k, v in in_maps[c].items():
                    core.tensor(k)[:] = v
            msim.simulate()
            shard_outs = [msim.cores[c].tensor("out").copy()
                          for c in range(n_cores)]
        res = None
    else:
        res = run_bass_kernel_spmd(nc, in_maps, core_ids=list(range(n_cores)))
        shard_outs = [res.results[c]["out"] for c in range(n_cores)]

    # un-permute
    out = np.zeros((p.N, D), dtype=np.float32)
    for c in range(n_cores):
        nodes = p.bin_nodes[c * p.NBPC:(c + 1) * p.NBPC].reshape(-1)
        valid = nodes >= 0
        out[nodes[valid]] = shard_outs[c][valid]
    kernel.last_results = res
    return out



# revision 10
# speedup vs baseline: 1.0774x; 1.0774x over previous
"""GATv2 (3-layer, H=1, D=128) on 8 Trainium2 NeuronCores via Bass/Tile.

Self-contained: host preprocessing (bin packing, edge tiling, int16 gather
indices), bass program builder, and the `kernel(**inputs)` entry point.

Strategy (dst-node sharding, uniform SPMD program):
  - Permute nodes into bins of 128 dst rows, 2-D balanced by (low,high)
    in-degree so every bin has exactly TL low + TH high edge tiles of 128.
  - Per layer: each core computes xl/xr for its shard (bf16), AllGather of
    xl -> xl_full in every core's HBM.
  - Per bin: dma_gather source features u[e,d] (bf16, 256B rows); per edge
    tile: PE transpose u -> wT psum, += xr^T-onehot matmul (U streamed from
    HBM); ACT LeakyRelu(+bl+br bias); score = lreluT^T @ att (PE);
    exp (ACT); denom/agg via one-hot matmuls accumulated in PSUM; epilogue
    recip + bias + relu (+ log_softmax on the last layer).
"""
import numpy as np
import ml_dtypes

import concourse.bacc as bacc
import concourse.mybir as mybir
import concourse.tile as tile
from concourse.bass_utils import run_bass_kernel_spmd

F32 = mybir.dt.float32
BF16 = mybir.dt.bfloat16
I16 = mybir.dt.int16
SLOPE = 0.2
D = 128
LOWCAP = 32768
DUMMY_DCOL = 200.0  # never matches iota 0..127 -> zero one-hot column


# ----------------------------------------------------------------------------
# Host preprocessing
# ----------------------------------------------------------------------------

def _pack_bins(n_bins, node_ids, w_low, w_high, cap=128):
    order = np.argsort(-(w_low + w_high), kind="stable")
    bins = [[] for _ in range(n_bins)]
    sums = np.zeros((n_bins, 2))
    counts = np.zeros(n_bins, dtype=np.int64)
    tgt_l = max(w_low.sum() / n_bins, 1.0)
    tgt_h = max(w_high.sum() / n_bins, 1.0)
    for idx in order:
        load = (sums[:, 0] + w_low[idx]) / tgt_l + (sums[:, 1] + w_high[idx]) / tgt_h
        load = np.where(counts >= cap, np.inf, load)
        b = int(np.argmin(load))
        bins[b].append(node_ids[idx])
        sums[b, 0] += w_low[idx]
        sums[b, 1] += w_high[idx]
        counts[b] += 1
    return bins, sums


def _chunks(n, cap=12):
    out = []
    while n > 0:
        c = min(n, cap)
        out.append(c)
        n -= c
    return out


class Plan:
    pass


def host_prep(x, edge_index, n_cores):
    """Returns Plan with per-core input tensors + structural constants."""
    N = x.shape[0]
    E = edge_index.shape[1]
    src = np.concatenate([np.asarray(edge_index[0], dtype=np.int64),
                          np.arange(N, dtype=np.int64)])
    dst = np.concatenate([np.asarray(edge_index[1], dtype=np.int64),
                          np.arange(N, dtype=np.int64)])

    NB = int(np.ceil(N / 128.0))
    NB = int(np.ceil(NB / n_cores)) * n_cores
    NPAD = NB * 128
    assert NPAD <= 65536
    lowcap = min(LOWCAP, NPAD)  # small configs: everything "low"
    hibase = NPAD - lowcap

    deg = np.bincount(dst, minlength=N).astype(np.float64)
    if NPAD > lowcap:
        # pass 1: balance total degree; defines the low/high node split
        bins1, _ = _pack_bins(NB, np.arange(N), deg, np.zeros(N))
        n_low_bins = lowcap // 128
        low_nodes = np.concatenate(
            [np.asarray(bins1[i], dtype=np.int64) for i in range(n_low_bins)]
        ) if n_low_bins else np.zeros(0, dtype=np.int64)
        is_low = np.zeros(N, dtype=bool)
        is_low[low_nodes] = True
        deg_low = np.bincount(dst[is_low[src]], minlength=N).astype(np.float64)
        deg_high = deg - deg_low
        ln = np.flatnonzero(is_low)
        hn = np.flatnonzero(~is_low)
        bins_l, _ = _pack_bins(n_low_bins, ln, deg_low[ln], deg_high[ln])
        bins_h, _ = _pack_bins(NB - n_low_bins, hn, deg_low[hn], deg_high[hn])
        bins = bins_l + bins_h
    else:
        bins, _ = _pack_bins(NB, np.arange(N), deg, np.zeros(N))
        is_low = np.ones(N, dtype=bool)

    # permutation: pos[old] = new index; bins padded with virtual rows
    pos = np.full(N, -1, dtype=np.int64)
    bin_nodes = np.full((NB, 128), -1, dtype=np.int64)
    for b, nodes in enumerate(bins):
        nodes = np.asarray(nodes, dtype=np.int64)
        bin_nodes[b, :len(nodes)] = nodes
        pos[nodes] = b * 128 + np.arange(len(nodes))

    # per-edge: bin, dst_local, low/high, gather idx
    e_pos_dst = pos[dst]
    e_bin = e_pos_dst // 128
    e_row = e_pos_dst % 128
    e_pos_src = pos[src]
    e_is_low = e_pos_src < lowcap

    # group edges per (bin, range)
    # sort by (bin, is_low desc) so lows first
    order = np.lexsort((~e_is_low, e_bin))
    sb, srow, sidx, slow = e_bin[order], e_row[order], e_pos_src[order], e_is_low[order]

    nl = np.zeros(NB, dtype=np.int64)
    nh = np.zeros(NB, dtype=np.int64)
    np.add.at(nl, e_bin[e_is_low], 1)
    np.add.at(nh, e_bin[~e_is_low], 1)
    # virtual rows get a self-edge in the low group
    nvirt = (bin_nodes < 0).sum(axis=1)
    TL = int(np.ceil((nl + nvirt).max() / 128.0))
    TH = int(np.ceil(nh.max() / 128.0)) if NPAD > lowcap else 0
    TPB = TL + TH
    low_calls = _chunks(TL)
    high_calls = _chunks(TH)
    calls_per_bin = [(c, 0) for c in low_calls] + [(c, 1) for c in high_calls]

    # slot arrays per bin: idx (int16), dcol (float), U one-hot
    slot_idx = np.zeros((NB, TPB * 128), dtype=np.int64)
    slot_dcol = np.full((NB, TPB * 128), DUMMY_DCOL, dtype=np.float64)
    # fill real edges
    # low edges occupy slots [0, nl), high edges [TL*128, TL*128+nh)
    starts = np.searchsorted(sb, np.arange(NB))
    ends = np.searchsorted(sb, np.arange(NB), side="right")
    for b in range(NB):
        s, e = starts[b], ends[b]
        lo_cnt = int(np.count_nonzero(slow[s:e]))
        hi_cnt = (e - s) - lo_cnt
        assert lo_cnt <= TL * 128 - nvirt[b] and hi_cnt <= TH * 128
        slot_idx[b, :lo_cnt] = sidx[s:s + lo_cnt]
        slot_dcol[b, :lo_cnt] = srow[s:s + lo_cnt]
        # virtual self-edges right after real low edges
        nv = int(nvirt[b])
        if nv:
            vrows = np.flatnonzero(bin_nodes[b] < 0)
            slot_idx[b, lo_cnt:lo_cnt + nv] = 0
            slot_dcol[b, lo_cnt:lo_cnt + nv] = vrows
        if hi_cnt:
            slot_idx[b, TL * 128:TL * 128 + hi_cnt] = sidx[s + lo_cnt:e] - hibase
            slot_dcol[b, TL * 128:TL * 128 + hi_cnt] = srow[s + lo_cnt:e]
    assert slot_idx.min() >= 0 and slot_idx.max() < 32768

    # wrapped int16 gather indices per call, replicated to 8 groups of 16
    def wrap(idx_flat):
        ni = len(idx_flat)
        a = idx_flat.astype(np.int16).reshape(ni // 16, 16).T  # [16, ni/16]
        w = np.tile(a, (8, 1))  # [128, ni/16]
        return w

    core_gidx = []
    core_dcol = []
    core_U = []
    core_x = []
    NBPC = NB // n_cores
    for c in range(n_cores):
        gx = []
        for b in range(c * NBPC, (c + 1) * NBPC):
            off = 0
            for ntile, rng in calls_per_bin:
                lo = (0 if rng == 0 else TL * 128) + (off if rng == 0 else off - TL * 128)
                gx.append(wrap(slot_idx[b, off:off + ntile * 128]))
                off += ntile * 128
            assert off == TPB * 128
        core_gidx.append(np.concatenate(gx, axis=1))
        dc = slot_dcol[c * NBPC:(c + 1) * NBPC].reshape(NBPC * TPB, 128)
        # dcol layout [128 partitions(e), NBPC*TPB]
        core_dcol.append(
            np.ascontiguousarray(dc.T).astype(ml_dtypes.bfloat16))
        # U one-hot [128 i, tiles*128 e]
        dloc = slot_dcol[c * NBPC:(c + 1) * NBPC].reshape(-1)  # [NBPC*TPB*128]
        U = (dloc[None, :] == np.arange(128)[:, None]).astype(ml_dtypes.bfloat16)
        core_U.append(np.ascontiguousarray(U))
        # x shard in permuted order (virtual rows zero)
        xs = np.zeros((NBPC * 128, D), dtype=np.float32)
        nodes = bin_nodes[c * NBPC:(c + 1) * NBPC].reshape(-1)
        valid = nodes >= 0
        xs[valid] = np.asarray(x)[nodes[valid]]
        core_x.append(xs)

    p = Plan()
    p.N, p.E, p.NB, p.NPAD, p.NBPC = N, E, NB, NPAD, NBPC
    p.TL, p.TH, p.TPB = TL, TH, TPB
    p.calls_per_bin = calls_per_bin
    p.lowcap, p.hibase = lowcap, hibase
    p.n_cores = n_cores
    p.core_gidx, p.core_dcol, p.core_U, p.core_x = (
        core_gidx, core_dcol, core_U, core_x)
    p.bin_nodes = bin_nodes
    p.pos = pos
    p.idx_cols = core_gidx[0].shape[1]
    return p


def host_weights(Wl, bl, Wr, br, att, bias, L):
    """Shared (replicated) weight tensors."""
    w = {}
    w["wl"] = np.ascontiguousarray(
        np.concatenate([np.asarray(Wl[l], np.float32) for l in range(L)], axis=1)
    ).astype(ml_dtypes.bfloat16)  # [128, L*128]
    w["wr"] = np.ascontiguousarray(
        np.concatenate([np.asarray(Wr[l], np.float32) for l in range(L)], axis=1)
    ).astype(ml_dtypes.bfloat16)
    w["att"] = np.ascontiguousarray(
        np.stack([np.asarray(att[l, 0], np.float32) for l in range(L)], axis=1)
    ).astype(ml_dtypes.bfloat16)  # [128, L]
    w["att2"] = np.ascontiguousarray(
        np.stack([np.asarray(-0.2 * att[l, 0], np.float32) for l in range(L)],
                 axis=1)).astype(ml_dtypes.bfloat16)  # [128, L]
    # lrelu bias = bl + br per layer, [128, L] f32
    w["lrb"] = np.ascontiguousarray(
        np.stack([np.asarray(bl[l] + br[l], np.float32) for l in range(L)], axis=1))
    w["lrbn"] = np.ascontiguousarray(-w["lrb"])
    # epilogue bias = bl + bias per layer, replicated [128, L*128] f32
    be = np.concatenate(
        [np.tile((np.asarray(bl[l] + bias[l], np.float32))[None, :], (128, 1))
         for l in range(L)], axis=1)
    w["be"] = np.ascontiguousarray(be)
    ident = np.eye(128, dtype=np.float32)
    w["idf"] = ident
    w["idb"] = ident.astype(ml_dtypes.bfloat16)
    w["iota"] = np.tile(np.arange(128, dtype=np.float32)[None, :], (128, 4)
                        ).astype(ml_dtypes.bfloat16)  # [128, 4*128]
    return w


# ----------------------------------------------------------------------------
# Device program
# ----------------------------------------------------------------------------

def build_nc(p, L=3):
    NBPC, TL, TH, TPB = p.NBPC, p.TL, p.TH, p.TPB
    NSH = NBPC * 128          # shard rows
    n_cores = p.n_cores
    NPAD = p.NPAD

    nc = bacc.Bacc("TRN2", target_bir_lowering=False, debug=False,
                   num_devices=n_cores, num_swdge_queues=4)

    # --- I/O ---
    x_in = nc.dram_tensor("x_shard", [NSH, D], F32, kind="ExternalInput")
    gidx_in = nc.dram_tensor("gidx", [128, p.idx_cols], I16, kind="ExternalInput")
    dcol_in = nc.dram_tensor("dcol", [128, NBPC * TPB], BF16, kind="ExternalInput")
    U_in = nc.dram_tensor("umat", [128, NBPC * TPB * 128], BF16,
                          kind="ExternalInput")
    wl_in = nc.dram_tensor("wl", [128, L * 128], BF16, kind="ExternalInput")
    wr_in = nc.dram_tensor("wr", [128, L * 128], BF16, kind="ExternalInput")
    att_in = nc.dram_tensor("att", [128, L], BF16, kind="ExternalInput")
    att2_in = nc.dram_tensor("att2", [128, L], BF16, kind="ExternalInput")
    lrb_in = nc.dram_tensor("lrb", [128, L], F32, kind="ExternalInput")
    lrbn_in = nc.dram_tensor("lrbn", [128, L], F32, kind="ExternalInput")
    be_in = nc.dram_tensor("be", [128, L * 128], F32, kind="ExternalInput")
    idf_in = nc.dram_tensor("idf", [128, 128], F32, kind="ExternalInput")
    idb_in = nc.dram_tensor("idb", [128, 128], BF16, kind="ExternalInput")
    iota_in = nc.dram_tensor("iota", [128, 4 * 128], BF16, kind="ExternalInput")
    out_t = nc.dram_tensor("out", [NSH, D], F32, kind="ExternalOutput")

    with tile.TileContext(nc) as tc:
        with (
            tc.tile_pool(name="const", bufs=1) as constp,
            tc.tile_pool(name="sb", bufs=2) as sb,
            tc.tile_pool(name="sb3", bufs=6) as sb3,
            tc.tile_pool(name="psum", bufs=2, space="PSUM") as ps,
            tc.tile_pool(name="dram", bufs=1, space="DRAM") as dram,
        ):
            # ---- constants / resident tensors ----
            gidx = constp.tile([128, p.idx_cols], I16)
            nc.sync.dma_start(gidx[:], gidx_in[:])
            dcol = constp.tile([128, NBPC * TPB], BF16)
            nc.sync.dma_start(dcol[:], dcol_in[:])
            wl = constp.tile([128, L * 128], BF16)
            nc.sync.dma_start(wl[:], wl_in[:])
            wr = constp.tile([128, L * 128], BF16)
            nc.sync.dma_start(wr[:], wr_in[:])
            att = constp.tile([128, L], BF16)
            nc.sync.dma_start(att[:], att_in[:])
            att2 = constp.tile([128, L], BF16)
            nc.sync.dma_start(att2[:], att2_in[:])
            lrb = constp.tile([128, L], F32)
            nc.sync.dma_start(lrb[:], lrb_in[:])
            lrbn = constp.tile([128, L], F32)
            nc.sync.dma_start(lrbn[:], lrbn_in[:])
            be = constp.tile([128, L * 128], F32)
            nc.sync.dma_start(be[:], be_in[:])
            idf = constp.tile([128, 128], F32)
            nc.sync.dma_start(idf[:], idf_in[:])
            idb = constp.tile([128, 128], BF16)
            nc.sync.dma_start(idb[:], idb_in[:])
            iota = constp.tile([128, 4 * 128], BF16)
            nc.sync.dma_start(iota[:], iota_in[:])
            # xr kept SBUF-resident per layer
            xr_res = constp.tile([128, NBPC * 128], BF16)

            # ---- DRAM scratch ----
            h_cur = dram.tile([NSH, D], F32)       # current layer input
            xl_shard = dram.tile([NSH, D], BF16)
            xl_full = dram.tile([NPAD, D], BF16)
            nc.sync.dma_start(h_cur[:], x_in[:])

            for l in range(L):
                # ======== Phase A: xl/xr shard + AllGather ========
                for t in range(NBPC):
                    h_t = sb.tile([128, 128], F32, tag="ht")
                    nc.sync.dma_start(h_t[:], h_cur[t * 128:(t + 1) * 128, :])
                    hT_ps = ps.tile([128, 128], F32, tag="wT")
                    nc.tensor.matmul(hT_ps[:], lhsT=h_t[:], rhs=idf[:],
                                     start=True, stop=True)
                    hT = sb.tile([128, 128], BF16, tag="hT")
                    nc.scalar.copy(hT[:], hT_ps[:])
                    xl_ps = ps.tile([128, 128], F32, tag="ag")
                    nc.tensor.matmul(xl_ps[:], lhsT=hT[:],
                                     rhs=wl[:, l * 128:(l + 1) * 128],
                                     start=True, stop=True)
                    xl_t = sb.tile([128, 128], BF16, tag="xlt")
                    nc.scalar.copy(xl_t[:], xl_ps[:])
                    nc.sync.dma_start(xl_shard[t * 128:(t + 1) * 128, :], xl_t[:])
                    xr_ps = ps.tile([128, 128], F32, tag="sc")
                    nc.tensor.matmul(xr_ps[:], lhsT=hT[:],
                                     rhs=wr[:, l * 128:(l + 1) * 128],
                                     start=True, stop=True)
                    nc.scalar.copy(xr_res[:, t * 128:(t + 1) * 128], xr_ps[:])

                nc.gpsimd.collective_compute(
                    "AllGather",
                    mybir.AluOpType.bypass,
                    replica_groups=[list(range(n_cores))],
                    ins=[xl_shard.opt()],
                    outs=[xl_full.opt()],
                )

                # ======== Phase B: per-bin edge processing ========
                idx_off = 0
                for b in range(NBPC):
                    u_bin = sb3.tile([128, TPB, 128], BF16, tag="u")
                    # gather calls
                    tile_off = 0
                    for ci, (ntile, rng) in enumerate(p.calls_per_bin):
                        ni = ntile * 128
                        base = 0 if rng == 0 else p.hibase
                        src_view = xl_full[base:base + p.lowcap, :]
                        nc.gpsimd.dma_gather(
                            out_ap=u_bin[:, tile_off:tile_off + ntile, :],
                            in_ap=src_view,
                            idxs_ap=gidx[:, idx_off:idx_off + ni // 16],
                            num_idxs=ni,
                            num_idxs_reg=ni,
                            elem_size=D,
                            queue_num=(b * len(p.calls_per_bin) + ci) % 4,
                            single_packet=False,
                        )
                        idx_off += ni // 16
                        tile_off += ntile
                    # U for this bin
                    U_bin = sb3.tile([128, TPB * 128], BF16, tag="U")
                    nc.sync.dma_start(
                        U_bin[:],
                        U_in[:, (b * TPB) * 128:((b + 1) * TPB) * 128])

                    xr_tile = xr_res[:, b * 128:(b + 1) * 128]
                    denom_ps = ps.tile([128, 1], F32, tag="dn")
                    agg_ps = ps.tile([128, 128], F32, tag="ag")

                    # batches of up to 4 tiles
                    bk_sizes = _chunks(TPB, 4)
                    t0 = 0
                    for bk in bk_sizes:
                        wT_ps = ps.tile([128, 4 * 128], F32, tag="wT")
                        for j in range(bk):
                            t = t0 + j
                            nc.tensor.matmul(
                                wT_ps[:, j * 128:(j + 1) * 128],
                                lhsT=u_bin[:, t, :], rhs=idb[:],
                                start=True, stop=False)
                            nc.tensor.matmul(
                                wT_ps[:, j * 128:(j + 1) * 128],
                                lhsT=xr_tile,
                                rhs=U_bin[:, t * 128:(t + 1) * 128],
                                start=False, stop=True)
                        lre = sb.tile([128, 4 * 128], BF16, tag="lre")
                        nc.scalar.activation(
                            lre[:, :bk * 128], wT_ps[:, :bk * 128],
                            mybir.ActivationFunctionType.Relu,
                            bias=lrb[:, l:l + 1], scale=1.0)
                        lrn = sb.tile([128, 4 * 128], BF16, tag="lrn")
                        nc.scalar.activation(
                            lrn[:, :bk * 128], wT_ps[:, :bk * 128],
                            mybir.ActivationFunctionType.Relu,
                            bias=lrbn[:, l:l + 1], scale=-1.0)
                        sc_ps = ps.tile([128, 4], F32, tag="sc")
                        for j in range(bk):
                            nc.tensor.matmul(
                                sc_ps[:, j:j + 1],
                                lhsT=lre[:, j * 128:(j + 1) * 128],
                                rhs=att[:, l:l + 1], start=True, stop=False)
                            nc.tensor.matmul(
                                sc_ps[:, j:j + 1],
                                lhsT=lrn[:, j * 128:(j + 1) * 128],
                                rhs=att2[:, l:l + 1], start=False, stop=True)
                        p8f = sb.tile([128, 4], F32, tag="p8f")
                        nc.scalar.activation(
                            p8f[:, :bk], sc_ps[:, :bk],
                            mybir.ActivationFunctionType.Exp)
                        p8 = sb.tile([128, 4], BF16, tag="p8")
                        nc.scalar.copy(p8[:, :bk], p8f[:, :bk])
                        A8 = sb.tile([128, 4, 128], BF16, tag="A8")
                        nc.vector.tensor_tensor(
                            out=A8[:, :bk, :],
                            in0=dcol[:, b * TPB + t0:b * TPB + t0 + bk]
                                .to_broadcast([128, bk, 128]),
                            in1=iota[:, :bk * 128].rearrange(
                                "p (b e) -> p b e", e=128),
                            op=mybir.AluOpType.is_equal)
                        for j in range(bk):
                            t = t0 + j
                            pu = sb.tile([128, 128], BF16, tag="pu")
                            nc.vector.tensor_tensor(
                                out=pu[:], in0=u_bin[:, t, :],
                                in1=p8f[:, j:j + 1].to_broadcast([128, 128]),
                                op=mybir.AluOpType.mult)
                            first = t == 0
                            last = t == TPB - 1
                            nc.tensor.matmul(
                                denom_ps[:], lhsT=A8[:, j, :],
                                rhs=p8[:, j:j + 1],
                                start=first, stop=last)
                            nc.tensor.matmul(
                                agg_ps[:], lhsT=A8[:, j, :], rhs=pu[:],
                                start=first, stop=last)
                        t0 += bk

                    # ---- epilogue ----
                    rec = sb.tile([128, 1], F32, tag="rec")
                    nc.vector.reciprocal(rec[:], denom_ps[:])
                    hp = sb.tile([128, 128], F32, tag="hp")
                    nc.vector.tensor_scalar_mul(hp[:], agg_ps[:], rec[:])
                    hb = sb.tile([128, 128], F32, tag="hb")
                    nc.vector.tensor_add(
                        hb[:], hp[:], be[:, l * 128:(l + 1) * 128])
                    if l < L - 1:
                        hn = sb.tile([128, 128], F32, tag="hn")
                        nc.scalar.activation(
                            hn[:], hb[:], mybir.ActivationFunctionType.Relu)
                        nc.sync.dma_start(
                            h_cur[b * 128:(b + 1) * 128, :], hn[:])
                    else:
                        z = sb.tile([128, 128], F32, tag="hn")
                        nc.scalar.activation(
                            z[:], hb[:], mybir.ActivationFunctionType.Relu)
                        m = sb.tile([128, 1], F32, tag="m")
                        nc.vector.tensor_reduce(
                            m[:], z[:], axis=mybir.AxisListType.X,
                            op=mybir.AluOpType.max)
                        zc = sb.tile([128, 128], F32, tag="zc")
                        nc.vector.tensor_scalar(
                            zc[:], z[:], scalar1=m[:], scalar2=None,
                            op0=mybir.AluOpType.subtract)
                        ez = sb.tile([128, 128], F32, tag="ez")
                        nc.scalar.activation(
                            ez[:], zc[:], mybir.ActivationFunctionType.Exp)
                        s = sb.tile([128, 1], F32, tag="s")
                        nc.vector.tensor_reduce(
                            s[:], ez[:], axis=mybir.AxisListType.X,
                            op=mybir.AluOpType.add)
                        ls = sb.tile([128, 1], F32, tag="ls")
                        nc.scalar.activation(
                            ls[:], s[:], mybir.ActivationFunctionType.Ln)
                        o = sb.tile([128, 128], F32, tag="o")
                        nc.vector.tensor_scalar(
                            o[:], zc[:], scalar1=ls[:], scalar2=None,
                            op0=mybir.AluOpType.subtract)
                        nc.sync.dma_start(
                            out_t[b * 128:(b + 1) * 128, :], o[:])

    nc.compile()
    return nc


# ----------------------------------------------------------------------------
# Entry point
# ----------------------------------------------------------------------------

def kernel(x, edge_index, Wl, bl, Wr, br, att, bias):
    n_cores = 8
    _sim = False
    x = np.asarray(x)
    edge_index = np.asarray(edge_index)
    L = np.asarray(Wl).shape[0]
    p = host_prep(x, edge_index, n_cores)
    w = host_weights(np.asarray(Wl), np.asarray(bl), np.asarray(Wr),
                     np.asarray(br), np.asarray(att), np.asarray(bias), L)
    nc = build_nc(p, L=L)

    in_maps = []
    for c in range(n_cores):
        m = {
            "x_shard": p.core_x[c],
            "gidx": p.core_gidx[c],
            "dcol": p.core_dcol[c],
            "umat": p.core_U[c],
            "wl": w["wl"], "wr": w["wr"], "att": w["att"],
            "att2": w["att2"], "lrb": w["lrb"], "lrbn": w["lrbn"],
            "be": w["be"],
            "idf": w["idf"], "idb": w["idb"], "iota": w["iota"],
        }
        in_maps.append(m)

    if _sim:
        from concourse.bass_interp import CoreSim, MultiCoreSim
        if n_cores == 1:
            sims = [CoreSim(nc)]
            for k, v in in_maps[0].items():
                sims[0].tensor(k)[:] = v
            sims[0].simulate()
            shard_outs = [sims[0].tensor("out").copy()]
        else:
            msim = MultiCoreSim(nc, num_cores=n_cores, trace=False)
            for c, core in sorted(msim.cores.items()):
                for k, v in in_maps[c].items():
                    core.tensor(k)[:] = v
            msim.simulate()
            shard_outs = [msim.cores[c].tensor("out").copy()
                          for c in range(n_cores)]
        res = None
    else:
        res = run_bass_kernel_spmd(nc, in_maps, core_ids=list(range(n_cores)))
        shard_outs = [res.results[c]["out"] for c in range(n_cores)]

    # un-permute
    out = np.zeros((p.N, D), dtype=np.float32)
    for c in range(n_cores):
        nodes = p.bin_nodes[c * p.NBPC:(c + 1) * p.NBPC].reshape(-1)
        valid = nodes >= 0
        out[nodes[valid]] = shard_outs[c][valid]
    kernel.last_results = res
    return out



# revision 11
# speedup vs baseline: 1.0860x; 1.0080x over previous
"""GATv2 (3-layer, H=1, D=128) on 8 Trainium2 NeuronCores via Bass/Tile.

Self-contained: host preprocessing (bin packing, edge tiling, int16 gather
indices), bass program builder, and the `kernel(**inputs)` entry point.

Strategy (dst-node sharding, uniform SPMD program):
  - Permute nodes into bins of 128 dst rows, 2-D balanced by (low,high)
    in-degree so every bin has exactly TL low + TH high edge tiles of 128.
  - Per layer: each core computes xl/xr for its shard (bf16), AllGather of
    xl -> xl_full in every core's HBM.
  - Per bin: dma_gather source features u[e,d] (bf16, 256B rows); per edge
    tile: PE transpose u -> wT psum, += xr^T-onehot matmul (U streamed from
    HBM); ACT LeakyRelu(+bl+br bias); score = lreluT^T @ att (PE);
    exp (ACT); denom/agg via one-hot matmuls accumulated in PSUM; epilogue
    recip + bias + relu (+ log_softmax on the last layer).
"""
import numpy as np
import ml_dtypes

import concourse.bacc as bacc
import concourse.mybir as mybir
import concourse.tile as tile
from concourse.bass_utils import run_bass_kernel_spmd

F32 = mybir.dt.float32
BF16 = mybir.dt.bfloat16
I16 = mybir.dt.int16
SLOPE = 0.2
D = 128
LOWCAP = 32768
DUMMY_DCOL = 200.0  # never matches iota 0..127 -> zero one-hot column


# ----------------------------------------------------------------------------
# Host preprocessing
# ----------------------------------------------------------------------------

def _pack_bins(n_bins, node_ids, w_low, w_high, cap=128):
    order = np.argsort(-(w_low + w_high), kind="stable")
    bins = [[] for _ in range(n_bins)]
    sums = np.zeros((n_bins, 2))
    counts = np.zeros(n_bins, dtype=np.int64)
    tgt_l = max(w_low.sum() / n_bins, 1.0)
    tgt_h = max(w_high.sum() / n_bins, 1.0)
    for idx in order:
        load = (sums[:, 0] + w_low[idx]) / tgt_l + (sums[:, 1] + w_high[idx]) / tgt_h
        load = np.where(counts >= cap, np.inf, load)
        b = int(np.argmin(load))
        bins[b].append(node_ids[idx])
        sums[b, 0] += w_low[idx]
        sums[b, 1] += w_high[idx]
        counts[b] += 1
    return bins, sums


def _chunks(n, cap=12):
    out = []
    while n > 0:
        c = min(n, cap)
        out.append(c)
        n -= c
    return out


class Plan:
    pass


def host_prep(x, edge_index, n_cores):
    """Returns Plan with per-core input tensors + structural constants."""
    N = x.shape[0]
    E = edge_index.shape[1]
    src = np.concatenate([np.asarray(edge_index[0], dtype=np.int64),
                          np.arange(N, dtype=np.int64)])
    dst = np.concatenate([np.asarray(edge_index[1], dtype=np.int64),
                          np.arange(N, dtype=np.int64)])

    NB = int(np.ceil(N / 128.0))
    NB = int(np.ceil(NB / n_cores)) * n_cores
    NPAD = NB * 128
    assert NPAD <= 65536
    lowcap = min(LOWCAP, NPAD)  # small configs: everything "low"
    hibase = NPAD - lowcap

    deg = np.bincount(dst, minlength=N).astype(np.float64)
    if NPAD > lowcap:
        # pass 1: balance total degree; defines the low/high node split
        bins1, _ = _pack_bins(NB, np.arange(N), deg, np.zeros(N))
        n_low_bins = lowcap // 128
        low_nodes = np.concatenate(
            [np.asarray(bins1[i], dtype=np.int64) for i in range(n_low_bins)]
        ) if n_low_bins else np.zeros(0, dtype=np.int64)
        is_low = np.zeros(N, dtype=bool)
        is_low[low_nodes] = True
        deg_low = np.bincount(dst[is_low[src]], minlength=N).astype(np.float64)
        deg_high = deg - deg_low
        ln = np.flatnonzero(is_low)
        hn = np.flatnonzero(~is_low)
        bins_l, _ = _pack_bins(n_low_bins, ln, deg_low[ln], deg_high[ln])
        bins_h, _ = _pack_bins(NB - n_low_bins, hn, deg_low[hn], deg_high[hn])
        bins = bins_l + bins_h
    else:
        bins, _ = _pack_bins(NB, np.arange(N), deg, np.zeros(N))
        is_low = np.ones(N, dtype=bool)

    # permutation: pos[old] = new index; bins padded with virtual rows
    pos = np.full(N, -1, dtype=np.int64)
    bin_nodes = np.full((NB, 128), -1, dtype=np.int64)
    for b, nodes in enumerate(bins):
        nodes = np.asarray(nodes, dtype=np.int64)
        bin_nodes[b, :len(nodes)] = nodes
        pos[nodes] = b * 128 + np.arange(len(nodes))

    # per-edge: bin, dst_local, low/high, gather idx
    e_pos_dst = pos[dst]
    e_bin = e_pos_dst // 128
    e_row = e_pos_dst % 128
    e_pos_src = pos[src]
    e_is_low = e_pos_src < lowcap

    # group edges per (bin, range)
    # sort by (bin, is_low desc) so lows first
    order = np.lexsort((~e_is_low, e_bin))
    sb, srow, sidx, slow = e_bin[order], e_row[order], e_pos_src[order], e_is_low[order]

    nl = np.zeros(NB, dtype=np.int64)
    nh = np.zeros(NB, dtype=np.int64)
    np.add.at(nl, e_bin[e_is_low], 1)
    np.add.at(nh, e_bin[~e_is_low], 1)
    # virtual rows get a self-edge in the low group
    nvirt = (bin_nodes < 0).sum(axis=1)
    TL = int(np.ceil((nl + nvirt).max() / 128.0))
    TH = int(np.ceil(nh.max() / 128.0)) if NPAD > lowcap else 0
    TPB = TL + TH
    low_calls = _chunks(TL)
    high_calls = _chunks(TH)
    calls_per_bin = [(c, 0) for c in low_calls] + [(c, 1) for c in high_calls]

    # slot arrays per bin: idx (int16), dcol (float), U one-hot
    slot_idx = np.zeros((NB, TPB * 128), dtype=np.int64)
    slot_dcol = np.full((NB, TPB * 128), DUMMY_DCOL, dtype=np.float64)
    # fill real edges
    # low edges occupy slots [0, nl), high edges [TL*128, TL*128+nh)
    starts = np.searchsorted(sb, np.arange(NB))
    ends = np.searchsorted(sb, np.arange(NB), side="right")
    for b in range(NB):
        s, e = starts[b], ends[b]
        lo_cnt = int(np.count_nonzero(slow[s:e]))
        hi_cnt = (e - s) - lo_cnt
        assert lo_cnt <= TL * 128 - nvirt[b] and hi_cnt <= TH * 128
        slot_idx[b, :lo_cnt] = sidx[s:s + lo_cnt]
        slot_dcol[b, :lo_cnt] = srow[s:s + lo_cnt]
        # virtual self-edges right after real low edges
        nv = int(nvirt[b])
        if nv:
            vrows = np.flatnonzero(bin_nodes[b] < 0)
            slot_idx[b, lo_cnt:lo_cnt + nv] = 0
            slot_dcol[b, lo_cnt:lo_cnt + nv] = vrows
        if hi_cnt:
            slot_idx[b, TL * 128:TL * 128 + hi_cnt] = sidx[s + lo_cnt:e] - hibase
            slot_dcol[b, TL * 128:TL * 128 + hi_cnt] = srow[s + lo_cnt:e]
    assert slot_idx.min() >= 0 and slot_idx.max() < 32768

    # wrapped int16 gather indices per call, replicated to 8 groups of 16
    def wrap(idx_flat):
        ni = len(idx_flat)
        a = idx_flat.astype(np.int16).reshape(ni // 16, 16).T  # [16, ni/16]
        w = np.tile(a, (8, 1))  # [128, ni/16]
        return w

    core_gidx = []
    core_dcol = []
    core_U = []
    core_x = []
    NBPC = NB // n_cores
    for c in range(n_cores):
        gx = []
        for b in range(c * NBPC, (c + 1) * NBPC):
            off = 0
            for ntile, rng in calls_per_bin:
                lo = (0 if rng == 0 else TL * 128) + (off if rng == 0 else off - TL * 128)
                gx.append(wrap(slot_idx[b, off:off + ntile * 128]))
                off += ntile * 128
            assert off == TPB * 128
        core_gidx.append(np.concatenate(gx, axis=1))
        dc = slot_dcol[c * NBPC:(c + 1) * NBPC].reshape(NBPC * TPB, 128)
        # dcol layout [128 partitions(e), NBPC*TPB]
        core_dcol.append(
            np.ascontiguousarray(dc.T).astype(ml_dtypes.bfloat16))
        # U one-hot [128 i, tiles*128 e]
        dloc = slot_dcol[c * NBPC:(c + 1) * NBPC].reshape(-1)  # [NBPC*TPB*128]
        U = (dloc[None, :] == np.arange(128)[:, None]).astype(ml_dtypes.bfloat16)
        core_U.append(np.ascontiguousarray(U))
        # x shard in permuted order (virtual rows zero)
        xs = np.zeros((NBPC * 128, D), dtype=np.float32)
        nodes = bin_nodes[c * NBPC:(c + 1) * NBPC].reshape(-1)
        valid = nodes >= 0
        xs[valid] = np.asarray(x)[nodes[valid]]
        core_x.append(xs)

    p = Plan()
    p.N, p.E, p.NB, p.NPAD, p.NBPC = N, E, NB, NPAD, NBPC
    p.TL, p.TH, p.TPB = TL, TH, TPB
    p.calls_per_bin = calls_per_bin
    p.lowcap, p.hibase = lowcap, hibase
    p.n_cores = n_cores
    p.core_gidx, p.core_dcol, p.core_U, p.core_x = (
        core_gidx, core_dcol, core_U, core_x)
    p.bin_nodes = bin_nodes
    p.pos = pos
    p.idx_cols = core_gidx[0].shape[1]
    return p


def host_weights(Wl, bl, Wr, br, att, bias, L):
    """Shared (replicated) weight tensors."""
    w = {}
    w["wl"] = np.ascontiguousarray(
        np.concatenate([np.asarray(Wl[l], np.float32) for l in range(L)], axis=1)
    ).astype(ml_dtypes.bfloat16)  # [128, L*128]
    w["wr"] = np.ascontiguousarray(
        np.concatenate([np.asarray(Wr[l], np.float32) for l in range(L)], axis=1)
    ).astype(ml_dtypes.bfloat16)
    w["att"] = np.ascontiguousarray(
        np.stack([np.asarray(att[l, 0], np.float32) for l in range(L)], axis=1)
    ).astype(ml_dtypes.bfloat16)  # [128, L]
    w["att2"] = np.ascontiguousarray(
        np.stack([np.asarray(-0.2 * att[l, 0], np.float32) for l in range(L)],
                 axis=1)).astype(ml_dtypes.bfloat16)  # [128, L]
    # lrelu bias = bl + br per layer, [128, L] f32
    w["lrb"] = np.ascontiguousarray(
        np.stack([np.asarray(bl[l] + br[l], np.float32) for l in range(L)], axis=1))
    w["lrbn"] = np.ascontiguousarray(-w["lrb"])
    # epilogue bias = bl + bias per layer, replicated [128, L*128] f32
    be = np.concatenate(
        [np.tile((np.asarray(bl[l] + bias[l], np.float32))[None, :], (128, 1))
         for l in range(L)], axis=1)
    w["be"] = np.ascontiguousarray(be)
    ident = np.eye(128, dtype=np.float32)
    w["idf"] = ident
    w["idb"] = ident.astype(ml_dtypes.bfloat16)
    w["iota"] = np.tile(np.arange(128, dtype=np.float32)[None, :], (128, 4)
                        ).astype(ml_dtypes.bfloat16)  # [128, 4*128]
    return w


# ----------------------------------------------------------------------------
# Device program
# ----------------------------------------------------------------------------

def build_nc(p, L=3):
    NBPC, TL, TH, TPB = p.NBPC, p.TL, p.TH, p.TPB
    NSH = NBPC * 128          # shard rows
    n_cores = p.n_cores
    NPAD = p.NPAD

    nc = bacc.Bacc("TRN2", target_bir_lowering=False, debug=False,
                   num_devices=n_cores, num_swdge_queues=4)

    # --- I/O ---
    x_in = nc.dram_tensor("x_shard", [NSH, D], F32, kind="ExternalInput")
    gidx_in = nc.dram_tensor("gidx", [128, p.idx_cols], I16, kind="ExternalInput")
    dcol_in = nc.dram_tensor("dcol", [128, NBPC * TPB], BF16, kind="ExternalInput")
    U_in = nc.dram_tensor("umat", [128, NBPC * TPB * 128], BF16,
                          kind="ExternalInput")
    wl_in = nc.dram_tensor("wl", [128, L * 128], BF16, kind="ExternalInput")
    wr_in = nc.dram_tensor("wr", [128, L * 128], BF16, kind="ExternalInput")
    att_in = nc.dram_tensor("att", [128, L], BF16, kind="ExternalInput")
    att2_in = nc.dram_tensor("att2", [128, L], BF16, kind="ExternalInput")
    lrb_in = nc.dram_tensor("lrb", [128, L], F32, kind="ExternalInput")
    lrbn_in = nc.dram_tensor("lrbn", [128, L], F32, kind="ExternalInput")
    be_in = nc.dram_tensor("be", [128, L * 128], F32, kind="ExternalInput")
    idf_in = nc.dram_tensor("idf", [128, 128], F32, kind="ExternalInput")
    idb_in = nc.dram_tensor("idb", [128, 128], BF16, kind="ExternalInput")
    iota_in = nc.dram_tensor("iota", [128, 4 * 128], BF16, kind="ExternalInput")
    out_t = nc.dram_tensor("out", [NSH, D], F32, kind="ExternalOutput")

    with tile.TileContext(nc) as tc:
        with (
            tc.tile_pool(name="const", bufs=1) as constp,
            tc.tile_pool(name="sb", bufs=2) as sb,
            tc.tile_pool(name="sb3", bufs=6) as sb3,
            tc.tile_pool(name="psum", bufs=2, space="PSUM") as ps,
            tc.tile_pool(name="dram", bufs=1, space="DRAM") as dram,
        ):
            # ---- constants / resident tensors ----
            gidx = constp.tile([128, p.idx_cols], I16)
            nc.sync.dma_start(gidx[:], gidx_in[:])
            dcol = constp.tile([128, NBPC * TPB], BF16)
            nc.sync.dma_start(dcol[:], dcol_in[:])
            wl = constp.tile([128, L * 128], BF16)
            nc.sync.dma_start(wl[:], wl_in[:])
            wr = constp.tile([128, L * 128], BF16)
            nc.sync.dma_start(wr[:], wr_in[:])
            att = constp.tile([128, L], BF16)
            nc.sync.dma_start(att[:], att_in[:])
            att2 = constp.tile([128, L], BF16)
            nc.sync.dma_start(att2[:], att2_in[:])
            lrb = constp.tile([128, L], F32)
            nc.sync.dma_start(lrb[:], lrb_in[:])
            lrbn = constp.tile([128, L], F32)
            nc.sync.dma_start(lrbn[:], lrbn_in[:])
            be = constp.tile([128, L * 128], F32)
            nc.sync.dma_start(be[:], be_in[:])
            idf = constp.tile([128, 128], F32)
            nc.sync.dma_start(idf[:], idf_in[:])
            idb = constp.tile([128, 128], BF16)
            nc.sync.dma_start(idb[:], idb_in[:])
            iota = constp.tile([128, 4 * 128], BF16)
            nc.sync.dma_start(iota[:], iota_in[:])
            # xr kept SBUF-resident per layer
            xr_res = constp.tile([128, NBPC * 128], BF16)

            # ---- DRAM scratch ----
            h_cur = dram.tile([NSH, D], F32)       # current layer input
            xl_shard = dram.tile([NSH, D], BF16)
            xl_full = dram.tile([NPAD, D], BF16)
            nc.sync.dma_start(h_cur[:], x_in[:])

            for l in range(L):
                # ======== Phase A: xl/xr shard + AllGather ========
                for t in range(NBPC):
                    h_t = sb.tile([128, 128], F32, tag="ht")
                    nc.sync.dma_start(h_t[:], h_cur[t * 128:(t + 1) * 128, :])
                    hT_ps = ps.tile([128, 128], F32, tag="wT")
                    nc.tensor.matmul(hT_ps[:], lhsT=h_t[:], rhs=idf[:],
                                     start=True, stop=True)
                    hT = sb.tile([128, 128], BF16, tag="hT")
                    nc.scalar.copy(hT[:], hT_ps[:])
                    xl_ps = ps.tile([128, 128], F32, tag="ag")
                    nc.tensor.matmul(xl_ps[:], lhsT=hT[:],
                                     rhs=wl[:, l * 128:(l + 1) * 128],
                                     start=True, stop=True)
                    xl_t = sb.tile([128, 128], BF16, tag="xlt")
                    nc.scalar.copy(xl_t[:], xl_ps[:])
                    nc.sync.dma_start(xl_shard[t * 128:(t + 1) * 128, :], xl_t[:])
                    xr_ps = ps.tile([128, 128], F32, tag="sc")
                    nc.tensor.matmul(xr_ps[:], lhsT=hT[:],
                                     rhs=wr[:, l * 128:(l + 1) * 128],
                                     start=True, stop=True)
                    nc.scalar.copy(xr_res[:, t * 128:(t + 1) * 128], xr_ps[:])

                nc.gpsimd.collective_compute(
                    "AllGather",
                    mybir.AluOpType.bypass,
                    replica_groups=[list(range(n_cores))],
                    ins=[xl_shard.opt()],
                    outs=[xl_full.opt()],
                )

                # ======== Phase B: per-bin edge processing ========
                idx_off = 0
                for b in range(NBPC):
                    u_bin = sb3.tile([128, TPB, 128], BF16, tag="u")
                    # gather calls
                    tile_off = 0
                    for ci, (ntile, rng) in enumerate(p.calls_per_bin):
                        ni = ntile * 128
                        base = 0 if rng == 0 else p.hibase
                        src_view = xl_full[base:base + p.lowcap, :]
                        nc.gpsimd.dma_gather(
                            out_ap=u_bin[:, tile_off:tile_off + ntile, :],
                            in_ap=src_view,
                            idxs_ap=gidx[:, idx_off:idx_off + ni // 16],
                            num_idxs=ni,
                            num_idxs_reg=ni,
                            elem_size=D,
                            queue_num=(b * len(p.calls_per_bin) + ci) % 4,
                            single_packet=False,
                        )
                        idx_off += ni // 16
                        tile_off += ntile
                    # U for this bin
                    U_bin = sb3.tile([128, TPB * 128], BF16, tag="U")
                    nc.sync.dma_start(
                        U_bin[:],
                        U_in[:, (b * TPB) * 128:((b + 1) * TPB) * 128])

                    xr_tile = xr_res[:, b * 128:(b + 1) * 128]
                    da_ps = ps.tile([128, 129], F32, tag="da")

                    # batches of up to 4 tiles
                    bk_sizes = _chunks(TPB, 4)
                    t0 = 0
                    for bk in bk_sizes:
                        wT_ps = ps.tile([128, 4 * 128], F32, tag="wT")
                        # one wide xr@U matmul seeds the chunk (1 LDW), then
                        # per-tile transposes accumulate on top
                        nc.tensor.matmul(
                            wT_ps[:, :bk * 128],
                            lhsT=xr_tile,
                            rhs=U_bin[:, t0 * 128:(t0 + bk) * 128],
                            start=True, stop=False)
                        for j in range(bk):
                            t = t0 + j
                            nc.tensor.matmul(
                                wT_ps[:, j * 128:(j + 1) * 128],
                                lhsT=u_bin[:, t, :], rhs=idb[:],
                                start=False, stop=True)
                        lre = sb.tile([128, 4 * 128], BF16, tag="lre")
                        nc.scalar.activation(
                            lre[:, :bk * 128], wT_ps[:, :bk * 128],
                            mybir.ActivationFunctionType.Relu,
                            bias=lrb[:, l:l + 1], scale=1.0)
                        lrn = sb.tile([128, 4 * 128], BF16, tag="lrn")
                        nc.scalar.activation(
                            lrn[:, :bk * 128], wT_ps[:, :bk * 128],
                            mybir.ActivationFunctionType.Relu,
                            bias=lrbn[:, l:l + 1], scale=-1.0)
                        sc_ps = ps.tile([128, 4], F32, tag="sc")
                        for j in range(bk):
                            nc.tensor.matmul(
                                sc_ps[:, j:j + 1],
                                lhsT=lre[:, j * 128:(j + 1) * 128],
                                rhs=att[:, l:l + 1], start=True, stop=False)
                            nc.tensor.matmul(
                                sc_ps[:, j:j + 1],
                                lhsT=lrn[:, j * 128:(j + 1) * 128],
                                rhs=att2[:, l:l + 1], start=False, stop=True)
                        # pp4 per tile: [pad, p8, pu(128)] so denom+agg is
                        # one matmul streaming cols 1:130
                        pp4 = sb.tile([128, 4, 130], BF16, tag="pp4")
                        nc.scalar.activation(
                            pp4[:, :bk, 1:2],
                            sc_ps[:, :bk].rearrange("p (b o) -> p b o", o=1),
                            mybir.ActivationFunctionType.Exp)
                        A8 = sb.tile([128, 4, 128], BF16, tag="A8")
                        nc.vector.tensor_tensor(
                            out=A8[:, :bk, :],
                            in0=dcol[:, b * TPB + t0:b * TPB + t0 + bk]
                                .to_broadcast([128, bk, 128]),
                            in1=iota[:, :bk * 128].rearrange(
                                "p (b e) -> p b e", e=128),
                            op=mybir.AluOpType.is_equal)
                        nc.vector.tensor_tensor(
                            out=pp4[:, :bk, 2:130],
                            in0=u_bin[:, t0:t0 + bk, :],
                            in1=pp4[:, :bk, 1:2].to_broadcast([128, bk, 128]),
                            op=mybir.AluOpType.mult)
                        for j in range(bk):
                            t = t0 + j
                            nc.tensor.matmul(
                                da_ps[:], lhsT=A8[:, j, :],
                                rhs=pp4[:, j, 1:130],
                                start=(t == 0), stop=(t == TPB - 1))
                        t0 += bk

                    # ---- epilogue ----
                    rec = sb.tile([128, 1], F32, tag="rec")
                    nc.vector.reciprocal(rec[:], da_ps[:, 0:1])
                    hp = sb.tile([128, 128], F32, tag="hp")
                    nc.vector.tensor_scalar_mul(hp[:], da_ps[:, 1:129], rec[:])
                    hb = sb.tile([128, 128], F32, tag="hb")
                    nc.vector.tensor_add(
                        hb[:], hp[:], be[:, l * 128:(l + 1) * 128])
                    if l < L - 1:
                        hn = sb.tile([128, 128], F32, tag="hn")
                        nc.scalar.activation(
                            hn[:], hb[:], mybir.ActivationFunctionType.Relu)
                        nc.sync.dma_start(
                            h_cur[b * 128:(b + 1) * 128, :], hn[:])
                    else:
                        z = sb.tile([128, 128], F32, tag="hn")
                        nc.scalar.activation(
                            z[:], hb[:], mybir.ActivationFunctionType.Relu)
                        m = sb.tile([128, 1], F32, tag="m")
                        nc.vector.tensor_reduce(
                            m[:], z[:], axis=mybir.AxisListType.X,
                            op=mybir.AluOpType.max)
                        zc = sb.tile([128, 128], F32, tag="zc")
                        nc.vector.tensor_scalar(
                            zc[:], z[:], scalar1=m[:], scalar2=None,
                            op0=mybir.AluOpType.subtract)
                        ez = sb.tile([128, 128], F32, tag="ez")
                        nc.scalar.activation(
                            ez[:], zc[:], mybir.ActivationFunctionType.Exp)
                        s = sb.tile([128, 1], F32, tag="s")
                        nc.vector.tensor_reduce(
                            s[:], ez[:], axis=mybir.AxisListType.X,
                            op=mybir.AluOpType.add)
                        ls = sb.tile([128, 1], F32, tag="ls")
                        nc.scalar.activation(
                            ls[:], s[:], mybir.ActivationFunctionType.Ln)
                        o = sb.tile([128, 128], F32, tag="o")
                        nc.vector.tensor_scalar(
                            o[:], zc[:], scalar1=ls[:], scalar2=None,
                            op0=mybir.AluOpType.subtract)
                        nc.sync.dma_start(
                            out_t[b * 128:(b + 1) * 128, :], o[:])

    nc.compile()
    return nc


# ----------------------------------------------------------------------------
# Entry point
# ----------------------------------------------------------------------------

def kernel(x, edge_index, Wl, bl, Wr, br, att, bias):
    n_cores = 8
    _sim = False
    x = np.asarray(x)
    edge_index = np.asarray(edge_index)
    L = np.asarray(Wl).shape[0]
    p = host_prep(x, edge_index, n_cores)
    w = host_weights(np.asarray(Wl), np.asarray(bl), np.asarray(Wr),
                     np.asarray(br), np.asarray(att), np.asarray(bias), L)
    nc = build_nc(p, L=L)

    in_maps = []
    for c in range(n_cores):
        m = {
            "x_shard": p.core_x[c],
            "gidx": p.core_gidx[c],
            "dcol": p.core_dcol[c],
            "umat": p.core_U[c],
            "wl": w["wl"], "wr": w["wr"], "att": w["att"],
            "att2": w["att2"], "lrb": w["lrb"], "lrbn": w["lrbn"],
            "be": w["be"],
            "idf": w["idf"], "idb": w["idb"], "iota": w["iota"],
        }
        in_maps.append(m)

    if _sim:
        from concourse.bass_interp import CoreSim, MultiCoreSim
        if n_cores == 1:
            sims = [CoreSim(nc)]
            for k, v in in_maps[0].items():
                sims[0].tensor(k)[:] = v
            sims[0].simulate()
            shard_outs = [sims[0].tensor("out").copy()]
        else:
            msim = MultiCoreSim(nc, num_cores=n_cores, trace=False)
            for c, core in sorted(msim.cores.items()):
                for k, v in in_maps[c].items():
                    core.tensor(k)[:] = v
            msim.simulate()
            shard_outs = [msim.cores[c].tensor("out").copy()
                          for c in range(n_cores)]
        res = None
    else:
        res = run_bass_kernel_spmd(nc, in_maps, core_ids=list(range(n_cores)))
        shard_outs = [res.results[c]["out"] for c in range(n_cores)]

    # un-permute
    out = np.zeros((p.N, D), dtype=np.float32)
    for c in range(n_cores):
        nodes = p.bin_nodes[c * p.NBPC:(c + 1) * p.NBPC].reshape(-1)
        valid = nodes >= 0
        out[nodes[valid]] = shard_outs[c][valid]
    kernel.last_results = res
    return out



# revision 12
# speedup vs baseline: 1.3343x; 1.2286x over previous
"""GATv2 (3-layer, H=1, D=128) on 8 Trainium2 NeuronCores via Bass/Tile.

Self-contained: host preprocessing (bin packing, edge tiling, int16 gather
indices), bass program builder, and the `kernel(**inputs)` entry point.

Strategy (dst-node sharding, uniform SPMD program):
  - Permute nodes into bins of 128 dst rows, 2-D balanced by (low,high)
    in-degree so every bin has exactly TL low + TH high edge tiles of 128.
  - Per layer: each core computes xl/xr for its shard (bf16), AllGather of
    xl -> xl_full in every core's HBM.
  - Per bin: dma_gather source features u[e,d] (bf16, 256B rows); per edge
    tile: PE transpose u -> wT psum, += xr^T-onehot matmul (U streamed from
    HBM); ACT LeakyRelu(+bl+br bias); score = lreluT^T @ att (PE);
    exp (ACT); denom/agg via one-hot matmuls accumulated in PSUM; epilogue
    recip + bias + relu (+ log_softmax on the last layer).
"""
import numpy as np
import ml_dtypes

import concourse.bacc as bacc
import concourse.mybir as mybir
import concourse.tile as tile
from concourse.bass_utils import run_bass_kernel_spmd

F32 = mybir.dt.float32
BF16 = mybir.dt.bfloat16
I16 = mybir.dt.int16
SLOPE = 0.2
D = 128
LOWCAP = 32768
DUMMY_DCOL = 200.0  # never matches iota 0..127 -> zero one-hot column


# ----------------------------------------------------------------------------
# Host preprocessing
# ----------------------------------------------------------------------------

def _pack_bins(n_bins, node_ids, w_low, w_high, cap=128):
    order = np.argsort(-(w_low + w_high), kind="stable")
    bins = [[] for _ in range(n_bins)]
    sums = np.zeros((n_bins, 2))
    counts = np.zeros(n_bins, dtype=np.int64)
    tgt_l = max(w_low.sum() / n_bins, 1.0)
    tgt_h = max(w_high.sum() / n_bins, 1.0)
    for idx in order:
        load = (sums[:, 0] + w_low[idx]) / tgt_l + (sums[:, 1] + w_high[idx]) / tgt_h
        load = np.where(counts >= cap, np.inf, load)
        b = int(np.argmin(load))
        bins[b].append(node_ids[idx])
        sums[b, 0] += w_low[idx]
        sums[b, 1] += w_high[idx]
        counts[b] += 1
    return bins, sums


def _chunks(n, cap=12):
    out = []
    while n > 0:
        c = min(n, cap)
        out.append(c)
        n -= c
    return out


class Plan:
    pass


def host_prep(x, edge_index, n_cores):
    """Returns Plan with per-core input tensors + structural constants."""
    N = x.shape[0]
    E = edge_index.shape[1]
    src = np.concatenate([np.asarray(edge_index[0], dtype=np.int64),
                          np.arange(N, dtype=np.int64)])
    dst = np.concatenate([np.asarray(edge_index[1], dtype=np.int64),
                          np.arange(N, dtype=np.int64)])

    NB = int(np.ceil(N / 128.0))
    NB = int(np.ceil(NB / n_cores)) * n_cores
    NPAD = NB * 128
    assert NPAD <= 65536
    lowcap = min(LOWCAP, NPAD)  # small configs: everything "low"
    hibase = NPAD - lowcap

    deg = np.bincount(dst, minlength=N).astype(np.float64)
    if NPAD > lowcap:
        # pass 1: balance total degree; defines the low/high node split
        bins1, _ = _pack_bins(NB, np.arange(N), deg, np.zeros(N))
        n_low_bins = lowcap // 128
        low_nodes = np.concatenate(
            [np.asarray(bins1[i], dtype=np.int64) for i in range(n_low_bins)]
        ) if n_low_bins else np.zeros(0, dtype=np.int64)
        is_low = np.zeros(N, dtype=bool)
        is_low[low_nodes] = True
        deg_low = np.bincount(dst[is_low[src]], minlength=N).astype(np.float64)
        deg_high = deg - deg_low
        ln = np.flatnonzero(is_low)
        hn = np.flatnonzero(~is_low)
        bins_l, _ = _pack_bins(n_low_bins, ln, deg_low[ln], deg_high[ln])
        bins_h, _ = _pack_bins(NB - n_low_bins, hn, deg_low[hn], deg_high[hn])
        bins = bins_l + bins_h
    else:
        bins, _ = _pack_bins(NB, np.arange(N), deg, np.zeros(N))
        is_low = np.ones(N, dtype=bool)

    # permutation: pos[old] = new index; bins padded with virtual rows
    pos = np.full(N, -1, dtype=np.int64)
    bin_nodes = np.full((NB, 128), -1, dtype=np.int64)
    for b, nodes in enumerate(bins):
        nodes = np.asarray(nodes, dtype=np.int64)
        bin_nodes[b, :len(nodes)] = nodes
        pos[nodes] = b * 128 + np.arange(len(nodes))

    # per-edge: bin, dst_local, low/high, gather idx
    e_pos_dst = pos[dst]
    e_bin = e_pos_dst // 128
    e_row = e_pos_dst % 128
    e_pos_src = pos[src]
    e_is_low = e_pos_src < lowcap

    # group edges per (bin, range)
    # sort by (bin, is_low desc) so lows first
    order = np.lexsort((~e_is_low, e_bin))
    sb, srow, sidx, slow = e_bin[order], e_row[order], e_pos_src[order], e_is_low[order]

    nl = np.zeros(NB, dtype=np.int64)
    nh = np.zeros(NB, dtype=np.int64)
    np.add.at(nl, e_bin[e_is_low], 1)
    np.add.at(nh, e_bin[~e_is_low], 1)
    # virtual rows get a self-edge in the low group
    nvirt = (bin_nodes < 0).sum(axis=1)
    TL = int(np.ceil((nl + nvirt).max() / 128.0))
    TH = int(np.ceil(nh.max() / 128.0)) if NPAD > lowcap else 0
    TPB = TL + TH
    low_calls = _chunks(TL)
    high_calls = _chunks(TH)
    calls_per_bin = [(c, 0) for c in low_calls] + [(c, 1) for c in high_calls]

    # slot arrays per bin: idx (int16), dcol (float), U one-hot
    slot_idx = np.zeros((NB, TPB * 128), dtype=np.int64)
    slot_dcol = np.full((NB, TPB * 128), DUMMY_DCOL, dtype=np.float64)
    # fill real edges
    # low edges occupy slots [0, nl), high edges [TL*128, TL*128+nh)
    starts = np.searchsorted(sb, np.arange(NB))
    ends = np.searchsorted(sb, np.arange(NB), side="right")
    for b in range(NB):
        s, e = starts[b], ends[b]
        lo_cnt = int(np.count_nonzero(slow[s:e]))
        hi_cnt = (e - s) - lo_cnt
        assert lo_cnt <= TL * 128 - nvirt[b] and hi_cnt <= TH * 128
        slot_idx[b, :lo_cnt] = sidx[s:s + lo_cnt]
        slot_dcol[b, :lo_cnt] = srow[s:s + lo_cnt]
        # virtual self-edges right after real low edges
        nv = int(nvirt[b])
        if nv:
            vrows = np.flatnonzero(bin_nodes[b] < 0)
            slot_idx[b, lo_cnt:lo_cnt + nv] = 0
            slot_dcol[b, lo_cnt:lo_cnt + nv] = vrows
        if hi_cnt:
            slot_idx[b, TL * 128:TL * 128 + hi_cnt] = sidx[s + lo_cnt:e] - hibase
            slot_dcol[b, TL * 128:TL * 128 + hi_cnt] = srow[s + lo_cnt:e]
    assert slot_idx.min() >= 0 and slot_idx.max() < 32768

    # wrapped int16 gather indices per call, replicated to 8 groups of 16
    def wrap(idx_flat):
        ni = len(idx_flat)
        a = idx_flat.astype(np.int16).reshape(ni // 16, 16).T  # [16, ni/16]
        w = np.tile(a, (8, 1))  # [128, ni/16]
        return w

    core_gidx = []
    core_dcol = []
    core_U = []
    core_x = []
    NBPC = NB // n_cores
    for c in range(n_cores):
        gx = []
        for b in range(c * NBPC, (c + 1) * NBPC):
            off = 0
            for ntile, rng in calls_per_bin:
                lo = (0 if rng == 0 else TL * 128) + (off if rng == 0 else off - TL * 128)
                gx.append(wrap(slot_idx[b, off:off + ntile * 128]))
                off += ntile * 128
            assert off == TPB * 128
        core_gidx.append(np.concatenate(gx, axis=1))
        dc = slot_dcol[c * NBPC:(c + 1) * NBPC].reshape(NBPC * TPB, 128)
        # dcol layout [128 partitions(e), NBPC*TPB]
        core_dcol.append(
            np.ascontiguousarray(dc.T).astype(ml_dtypes.bfloat16))
        # U one-hot [128 i, tiles*128 e]
        dloc = slot_dcol[c * NBPC:(c + 1) * NBPC].reshape(-1)  # [NBPC*TPB*128]
        U = (dloc[None, :] == np.arange(128)[:, None]).astype(ml_dtypes.bfloat16)
        core_U.append(np.ascontiguousarray(U))
        # x shard in permuted order (virtual rows zero)
        xs = np.zeros((NBPC * 128, D), dtype=np.float32)
        nodes = bin_nodes[c * NBPC:(c + 1) * NBPC].reshape(-1)
        valid = nodes >= 0
        xs[valid] = np.asarray(x)[nodes[valid]]
        core_x.append(xs)

    p = Plan()
    p.N, p.E, p.NB, p.NPAD, p.NBPC = N, E, NB, NPAD, NBPC
    p.TL, p.TH, p.TPB = TL, TH, TPB
    p.calls_per_bin = calls_per_bin
    p.lowcap, p.hibase = lowcap, hibase
    p.n_cores = n_cores
    p.core_gidx, p.core_dcol, p.core_U, p.core_x = (
        core_gidx, core_dcol, core_U, core_x)
    p.bin_nodes = bin_nodes
    p.pos = pos
    p.idx_cols = core_gidx[0].shape[1]
    return p


def host_weights(Wl, bl, Wr, br, att, bias, L):
    """Shared (replicated) weight tensors."""
    w = {}
    w["wl"] = np.ascontiguousarray(
        np.concatenate([np.asarray(Wl[l], np.float32) for l in range(L)], axis=1)
    ).astype(ml_dtypes.bfloat16)  # [128, L*128]
    w["wr"] = np.ascontiguousarray(
        np.concatenate([np.asarray(Wr[l], np.float32) for l in range(L)], axis=1)
    ).astype(ml_dtypes.bfloat16)
    w["att"] = np.ascontiguousarray(
        np.stack([np.asarray(att[l, 0], np.float32) for l in range(L)], axis=1)
    ).astype(ml_dtypes.bfloat16)  # [128, L]
    w["att2"] = np.ascontiguousarray(
        np.stack([np.asarray(-0.2 * att[l, 0], np.float32) for l in range(L)],
                 axis=1)).astype(ml_dtypes.bfloat16)  # [128, L]
    # lrelu bias = bl + br per layer, [128, L] f32
    w["lrb"] = np.ascontiguousarray(
        np.stack([np.asarray(bl[l] + br[l], np.float32) for l in range(L)], axis=1))
    w["lrbn"] = np.ascontiguousarray(-w["lrb"])
    # epilogue bias = bl + bias per layer, replicated [128, L*128] f32
    be = np.concatenate(
        [np.tile((np.asarray(bl[l] + bias[l], np.float32))[None, :], (128, 1))
         for l in range(L)], axis=1)
    w["be"] = np.ascontiguousarray(be)
    ident = np.eye(128, dtype=np.float32)
    w["idf"] = ident
    w["idb"] = ident.astype(ml_dtypes.bfloat16)
    w["iota"] = np.tile(np.arange(128, dtype=np.float32)[None, :], (128, 4)
                        ).astype(ml_dtypes.bfloat16)  # [128, 4*128]
    return w


# ----------------------------------------------------------------------------
# Device program
# ----------------------------------------------------------------------------

def build_nc(p, L=3):
    NBPC, TL, TH, TPB = p.NBPC, p.TL, p.TH, p.TPB
    NSH = NBPC * 128          # shard rows
    n_cores = p.n_cores
    NPAD = p.NPAD

    nc = bacc.Bacc("TRN2", target_bir_lowering=False, debug=False,
                   num_devices=n_cores, num_swdge_queues=4)

    # --- I/O ---
    x_in = nc.dram_tensor("x_shard", [NSH, D], F32, kind="ExternalInput")
    gidx_in = nc.dram_tensor("gidx", [128, p.idx_cols], I16, kind="ExternalInput")
    dcol_in = nc.dram_tensor("dcol", [128, NBPC * TPB], BF16, kind="ExternalInput")
    U_in = nc.dram_tensor("umat", [128, NBPC * TPB * 128], BF16,
                          kind="ExternalInput")
    wl_in = nc.dram_tensor("wl", [128, L * 128], BF16, kind="ExternalInput")
    wr_in = nc.dram_tensor("wr", [128, L * 128], BF16, kind="ExternalInput")
    att_in = nc.dram_tensor("att", [128, L], BF16, kind="ExternalInput")
    att2_in = nc.dram_tensor("att2", [128, L], BF16, kind="ExternalInput")
    lrb_in = nc.dram_tensor("lrb", [128, L], F32, kind="ExternalInput")
    lrbn_in = nc.dram_tensor("lrbn", [128, L], F32, kind="ExternalInput")
    be_in = nc.dram_tensor("be", [128, L * 128], F32, kind="ExternalInput")
    idf_in = nc.dram_tensor("idf", [128, 128], F32, kind="ExternalInput")
    idb_in = nc.dram_tensor("idb", [128, 128], BF16, kind="ExternalInput")
    iota_in = nc.dram_tensor("iota", [128, 4 * 128], BF16, kind="ExternalInput")
    out_t = nc.dram_tensor("out", [NSH, D], F32, kind="ExternalOutput")

    with tile.TileContext(nc) as tc:
        with (
            tc.tile_pool(name="const", bufs=1) as constp,
            tc.tile_pool(name="sb", bufs=2) as sb,
            tc.tile_pool(name="sb3", bufs=6) as sb3,
            tc.tile_pool(name="psum", bufs=2, space="PSUM") as ps,
            tc.tile_pool(name="dram", bufs=1, space="DRAM") as dram,
        ):
            # ---- constants / resident tensors ----
            gidx = constp.tile([128, p.idx_cols], I16)
            nc.sync.dma_start(gidx[:], gidx_in[:])
            dcol = constp.tile([128, NBPC * TPB], BF16)
            nc.sync.dma_start(dcol[:], dcol_in[:])
            wl = constp.tile([128, L * 128], BF16)
            nc.sync.dma_start(wl[:], wl_in[:])
            wr = constp.tile([128, L * 128], BF16)
            nc.sync.dma_start(wr[:], wr_in[:])
            att = constp.tile([128, L], BF16)
            nc.sync.dma_start(att[:], att_in[:])
            att2 = constp.tile([128, L], BF16)
            nc.sync.dma_start(att2[:], att2_in[:])
            lrb = constp.tile([128, L], F32)
            nc.sync.dma_start(lrb[:], lrb_in[:])
            lrbn = constp.tile([128, L], F32)
            nc.sync.dma_start(lrbn[:], lrbn_in[:])
            be = constp.tile([128, L * 128], F32)
            nc.sync.dma_start(be[:], be_in[:])
            idf = constp.tile([128, 128], F32)
            nc.sync.dma_start(idf[:], idf_in[:])
            idb = constp.tile([128, 128], BF16)
            nc.sync.dma_start(idb[:], idb_in[:])
            iota = constp.tile([128, 4 * 128], BF16)
            nc.sync.dma_start(iota[:], iota_in[:])
            # xr kept SBUF-resident per layer
            xr_res = constp.tile([128, NBPC * 128], BF16)

            # ---- DRAM scratch ----
            h_cur = dram.tile([NSH, D], F32)       # current layer input
            xl_shard = dram.tile([NSH, D], BF16)
            xl_full = dram.tile([NPAD, D], BF16)
            nc.sync.dma_start(h_cur[:], x_in[:])

            for l in range(L):
                # ======== Phase A: xl/xr shard + AllGather ========
                for t in range(NBPC):
                    h_t = sb.tile([128, 128], F32, tag="ht")
                    nc.sync.dma_start(h_t[:], h_cur[t * 128:(t + 1) * 128, :])
                    hT_ps = ps.tile([128, 128], F32, tag="wT")
                    nc.tensor.matmul(hT_ps[:], lhsT=h_t[:], rhs=idf[:],
                                     start=True, stop=True)
                    hT = sb.tile([128, 128], BF16, tag="hT")
                    nc.scalar.copy(hT[:], hT_ps[:])
                    xl_ps = ps.tile([128, 128], F32, tag="ag")
                    nc.tensor.matmul(xl_ps[:], lhsT=hT[:],
                                     rhs=wl[:, l * 128:(l + 1) * 128],
                                     start=True, stop=True)
                    xl_t = sb.tile([128, 128], BF16, tag="xlt")
                    nc.scalar.copy(xl_t[:], xl_ps[:])
                    nc.sync.dma_start(xl_shard[t * 128:(t + 1) * 128, :], xl_t[:])
                    xr_ps = ps.tile([128, 128], F32, tag="sc")
                    nc.tensor.matmul(xr_ps[:], lhsT=hT[:],
                                     rhs=wr[:, l * 128:(l + 1) * 128],
                                     start=True, stop=True)
                    nc.scalar.copy(xr_res[:, t * 128:(t + 1) * 128], xr_ps[:])

                nc.gpsimd.collective_compute(
                    "AllGather",
                    mybir.AluOpType.bypass,
                    replica_groups=[list(range(n_cores))],
                    ins=[xl_shard.opt()],
                    outs=[xl_full.opt()],
                )

                # ======== Phase B: per-bin edge processing ========
                idx_off = 0
                for b in range(NBPC):
                    u_bin = sb3.tile([128, TPB, 128], BF16, tag="u")
                    # gather calls
                    tile_off = 0
                    for ci, (ntile, rng) in enumerate(p.calls_per_bin):
                        ni = ntile * 128
                        base = 0 if rng == 0 else p.hibase
                        src_view = xl_full[base:base + p.lowcap, :]
                        nc.gpsimd.dma_gather(
                            out_ap=u_bin[:, tile_off:tile_off + ntile, :],
                            in_ap=src_view,
                            idxs_ap=gidx[:, idx_off:idx_off + ni // 16],
                            num_idxs=ni,
                            num_idxs_reg=ni,
                            elem_size=D,
                            queue_num=(b * len(p.calls_per_bin) + ci) % 4,
                            single_packet=False,
                        )
                        idx_off += ni // 16
                        tile_off += ntile
                    # U for this bin
                    U_bin = sb3.tile([128, TPB * 128], BF16, tag="U")
                    nc.sync.dma_start(
                        U_bin[:],
                        U_in[:, (b * TPB) * 128:((b + 1) * TPB) * 128])

                    xr_tile = xr_res[:, b * 128:(b + 1) * 128]
                    da_ps = ps.tile([128, 129], F32, tag="da")

                    # batches of up to 4 tiles
                    bk_sizes = _chunks(TPB, 4)
                    t0 = 0
                    for bk in bk_sizes:
                        wT_ps = ps.tile([128, 4 * 128], F32, tag="wT")
                        # one wide xr@U matmul seeds the chunk (1 LDW), then
                        # per-tile transposes accumulate on top
                        nc.tensor.matmul(
                            wT_ps[:, :bk * 128],
                            lhsT=xr_tile,
                            rhs=U_bin[:, t0 * 128:(t0 + bk) * 128],
                            start=True, stop=False)
                        for j in range(bk):
                            t = t0 + j
                            nc.tensor.matmul(
                                wT_ps[:, j * 128:(j + 1) * 128],
                                lhsT=u_bin[:, t, :], rhs=idb[:],
                                start=False, stop=True)
                        lre = sb.tile([128, 4 * 128], BF16, tag="lre")
                        nc.scalar.activation(
                            lre[:, :bk * 128], wT_ps[:, :bk * 128],
                            mybir.ActivationFunctionType.Relu,
                            bias=lrb[:, l:l + 1], scale=1.0)
                        lrn = sb.tile([128, 4 * 128], BF16, tag="lrn")
                        nc.scalar.activation(
                            lrn[:, :bk * 128], wT_ps[:, :bk * 128],
                            mybir.ActivationFunctionType.Relu,
                            bias=lrbn[:, l:l + 1], scale=-1.0)
                        sc_ps = ps.tile([128, 4], F32, tag="sc")
                        for j in range(bk):
                            nc.tensor.matmul(
                                sc_ps[:, j:j + 1],
                                lhsT=lre[:, j * 128:(j + 1) * 128],
                                rhs=att[:, l:l + 1], start=True, stop=False)
                            nc.tensor.matmul(
                                sc_ps[:, j:j + 1],
                                lhsT=lrn[:, j * 128:(j + 1) * 128],
                                rhs=att2[:, l:l + 1], start=False, stop=True)
                        # pp4 per tile: [pad, p8, pu(128)] so denom+agg is
                        # one matmul streaming cols 1:130
                        pp4 = sb.tile([128, 4, 130], BF16, tag="pp4")
                        nc.scalar.activation(
                            pp4[:, :bk, 1:2],
                            sc_ps[:, :bk].rearrange("p (b o) -> p b o", o=1),
                            mybir.ActivationFunctionType.Exp)
                        A8 = sb.tile([128, 4, 128], BF16, tag="A8")
                        nc.vector.tensor_tensor(
                            out=A8[:, :bk, :],
                            in0=dcol[:, b * TPB + t0:b * TPB + t0 + bk]
                                .to_broadcast([128, bk, 128]),
                            in1=iota[:, :bk * 128].rearrange(
                                "p (b e) -> p b e", e=128),
                            op=mybir.AluOpType.is_equal)
                        nc.vector.tensor_tensor(
                            out=pp4[:, :bk, 2:130],
                            in0=u_bin[:, t0:t0 + bk, :],
                            in1=pp4[:, :bk, 1:2].to_broadcast([128, bk, 128]),
                            op=mybir.AluOpType.mult)
                        for j in range(bk):
                            t = t0 + j
                            nc.tensor.matmul(
                                da_ps[:], lhsT=A8[:, j, :],
                                rhs=pp4[:, j, 1:130],
                                start=(t == 0), stop=(t == TPB - 1))
                        t0 += bk

                    # ---- epilogue ----
                    rec = sb.tile([128, 1], F32, tag="rec")
                    nc.vector.reciprocal(rec[:], da_ps[:, 0:1])
                    hp = sb.tile([128, 128], F32, tag="hp")
                    nc.vector.tensor_scalar_mul(hp[:], da_ps[:, 1:129], rec[:])
                    hb = sb.tile([128, 128], F32, tag="hb")
                    nc.vector.tensor_add(
                        hb[:], hp[:], be[:, l * 128:(l + 1) * 128])
                    if l < L - 1:
                        hn = sb.tile([128, 128], F32, tag="hn")
                        nc.scalar.activation(
                            hn[:], hb[:], mybir.ActivationFunctionType.Relu)
                        nc.sync.dma_start(
                            h_cur[b * 128:(b + 1) * 128, :], hn[:])
                    else:
                        z = sb.tile([128, 128], F32, tag="hn")
                        nc.scalar.activation(
                            z[:], hb[:], mybir.ActivationFunctionType.Relu)
                        m = sb.tile([128, 1], F32, tag="m")
                        nc.vector.tensor_reduce(
                            m[:], z[:], axis=mybir.AxisListType.X,
                            op=mybir.AluOpType.max)
                        mn = sb.tile([128, 1], F32, tag="mn")
                        nc.scalar.mul(mn[:], m[:], -1.0)
                        zc = sb.tile([128, 128], F32, tag="zc")
                        nc.scalar.activation(
                            zc[:], z[:],
                            mybir.ActivationFunctionType.Identity,
                            bias=mn[:], scale=1.0)
                        ez = sb.tile([128, 128], F32, tag="ez")
                        nc.scalar.activation(
                            ez[:], zc[:], mybir.ActivationFunctionType.Exp)
                        s = sb.tile([128, 1], F32, tag="s")
                        nc.vector.tensor_reduce(
                            s[:], ez[:], axis=mybir.AxisListType.X,
                            op=mybir.AluOpType.add)
                        ls = sb.tile([128, 1], F32, tag="ls")
                        nc.scalar.activation(
                            ls[:], s[:], mybir.ActivationFunctionType.Ln)
                        lsn = sb.tile([128, 1], F32, tag="lsn")
                        nc.scalar.mul(lsn[:], ls[:], -1.0)
                        o = sb.tile([128, 128], F32, tag="o")
                        nc.scalar.activation(
                            o[:], zc[:],
                            mybir.ActivationFunctionType.Identity,
                            bias=lsn[:], scale=1.0)
                        nc.sync.dma_start(
                            out_t[b * 128:(b + 1) * 128, :], o[:])

    nc.compile()
    return nc


# ----------------------------------------------------------------------------
# Entry point
# ----------------------------------------------------------------------------

def kernel(x, edge_index, Wl, bl, Wr, br, att, bias):
    n_cores = 8
    _sim = False
    x = np.asarray(x)
    edge_index = np.asarray(edge_index)
    L = np.asarray(Wl).shape[0]
    p = host_prep(x, edge_index, n_cores)
    w = host_weights(np.asarray(Wl), np.asarray(bl), np.asarray(Wr),
                     np.asarray(br), np.asarray(att), np.asarray(bias), L)
    nc = build_nc(p, L=L)

    in_maps = []
    for c in range(n_cores):
        m = {
            "x_shard": p.core_x[c],
            "gidx": p.core_gidx[c],
            "dcol": p.core_dcol[c],
            "umat": p.core_U[c],
            "wl": w["wl"], "wr": w["wr"], "att": w["att"],
            "att2": w["att2"], "lrb": w["lrb"], "lrbn": w["lrbn"],
            "be": w["be"],
            "idf": w["idf"], "idb": w["idb"], "iota": w["iota"],
        }
        in_maps.append(m)

    if _sim:
        from concourse.bass_interp import CoreSim, MultiCoreSim
        if n_cores == 1:
            sims = [CoreSim(nc)]
            for k, v in in_maps[0].items():
                sims[0].tensor(k)[:] = v
            sims[0].simulate()
            shard_outs = [sims[0].tensor("out").copy()]
        else:
            msim = MultiCoreSim(nc, num_cores=n_cores, trace=False)
            for c, core in sorted(msim.cores.items()):
                for k, v in in_maps[c].items():
                    core.tensor(k)[:] = v
            msim.simulate()
            shard_outs = [msim.cores[c].tensor("out").copy()
                          for c in range(n_cores)]
        res = None
    else:
        res = run_bass_kernel_spmd(nc, in_maps, core_ids=list(range(n_cores)))
        shard_outs = [res.results[c]["out"] for c in range(n_cores)]

    # un-permute
    out = np.zeros((p.N, D), dtype=np.float32)
    for c in range(n_cores):
        nodes = p.bin_nodes[c * p.NBPC:(c + 1) * p.NBPC].reshape(-1)
        valid = nodes >= 0
        out[nodes[valid]] = shard_outs[c][valid]
    kernel.last_results = res
    return out



# revision 13
# speedup vs baseline: 1.3560x; 1.0163x over previous
"""GATv2 (3-layer, H=1, D=128) on 8 Trainium2 NeuronCores via Bass/Tile.

Self-contained: host preprocessing (bin packing, edge tiling, int16 gather
indices), bass program builder, and the `kernel(**inputs)` entry point.

Strategy (dst-node sharding, uniform SPMD program):
  - Permute nodes into bins of 128 dst rows, 2-D balanced by (low,high)
    in-degree so every bin has exactly TL low + TH high edge tiles of 128.
  - Per layer: each core computes xl/xr for its shard (bf16), AllGather of
    xl -> xl_full in every core's HBM.
  - Per bin: dma_gather source features u[e,d] (bf16, 256B rows); per edge
    tile: PE transpose u -> wT psum, += xr^T-onehot matmul (U streamed from
    HBM); ACT LeakyRelu(+bl+br bias); score = lreluT^T @ att (PE);
    exp (ACT); denom/agg via one-hot matmuls accumulated in PSUM; epilogue
    recip + bias + relu (+ log_softmax on the last layer).
"""
import numpy as np
import ml_dtypes

import concourse.bacc as bacc
import concourse.mybir as mybir
import concourse.tile as tile
from concourse.bass_utils import run_bass_kernel_spmd

F32 = mybir.dt.float32
BF16 = mybir.dt.bfloat16
I16 = mybir.dt.int16
SLOPE = 0.2
D = 128
LOWCAP = 32768
DUMMY_DCOL = 200.0  # never matches iota 0..127 -> zero one-hot column


# ----------------------------------------------------------------------------
# Host preprocessing
# ----------------------------------------------------------------------------

def _pack_bins(n_bins, node_ids, w_low, w_high, cap=128,
               cap_low=np.inf, cap_high=np.inf):
    order = np.argsort(-(w_low + w_high), kind="stable")
    bins = [[] for _ in range(n_bins)]
    sums = np.zeros((n_bins, 2))
    counts = np.zeros(n_bins, dtype=np.int64)
    tgt_l = max(w_low.sum() / n_bins, 1.0)
    tgt_h = max(w_high.sum() / n_bins, 1.0)
    for idx in order:
        load = (sums[:, 0] + w_low[idx]) / tgt_l + (sums[:, 1] + w_high[idx]) / tgt_h
        load = np.where(counts >= cap, np.inf, load)
        load = np.where(sums[:, 0] + w_low[idx] > cap_low, np.inf, load)
        load = np.where(sums[:, 1] + w_high[idx] > cap_high, np.inf, load)
        if not np.isfinite(load).any():
            return None, None
        b = int(np.argmin(load))
        bins[b].append(node_ids[idx])
        sums[b, 0] += w_low[idx]
        sums[b, 1] += w_high[idx]
        counts[b] += 1
    return bins, sums


def _pack_bins_capped(n_bins, node_ids, w_low, w_high, cap=128):
    """Try progressively looser (cap_low, cap_high) slot caps so the max
    bin needs fewer 128-edge tiles; fall back to uncapped."""
    for cl, ch in ((1530, 764), (1530, 890), (1658, 890),
                   (np.inf, np.inf)):
        bins, sums = _pack_bins(n_bins, node_ids, w_low, w_high, cap=cap,
                                cap_low=cl, cap_high=ch)
        if bins is not None:
            return bins, sums
    raise AssertionError("unreachable")


def _chunks(n, cap=12):
    out = []
    while n > 0:
        c = min(n, cap)
        out.append(c)
        n -= c
    return out


class Plan:
    pass


def host_prep(x, edge_index, n_cores):
    """Returns Plan with per-core input tensors + structural constants."""
    N = x.shape[0]
    E = edge_index.shape[1]
    src = np.concatenate([np.asarray(edge_index[0], dtype=np.int64),
                          np.arange(N, dtype=np.int64)])
    dst = np.concatenate([np.asarray(edge_index[1], dtype=np.int64),
                          np.arange(N, dtype=np.int64)])

    NB = int(np.ceil(N / 128.0))
    NB = int(np.ceil(NB / n_cores)) * n_cores
    NPAD = NB * 128
    assert NPAD <= 65536
    lowcap = min(LOWCAP, NPAD)  # small configs: everything "low"
    hibase = NPAD - lowcap

    deg = np.bincount(dst, minlength=N).astype(np.float64)
    if NPAD > lowcap:
        # pass 1: balance total degree; defines the low/high node split
        bins1, _ = _pack_bins(NB, np.arange(N), deg, np.zeros(N))
        n_low_bins = lowcap // 128
        low_nodes = np.concatenate(
            [np.asarray(bins1[i], dtype=np.int64) for i in range(n_low_bins)]
        ) if n_low_bins else np.zeros(0, dtype=np.int64)
        is_low = np.zeros(N, dtype=bool)
        is_low[low_nodes] = True
        deg_low = np.bincount(dst[is_low[src]], minlength=N).astype(np.float64)
        deg_high = deg - deg_low
        ln = np.flatnonzero(is_low)
        hn = np.flatnonzero(~is_low)
        bins_l, _ = _pack_bins_capped(n_low_bins, ln, deg_low[ln],
                                      deg_high[ln])
        bins_h, _ = _pack_bins_capped(NB - n_low_bins, hn, deg_low[hn],
                                      deg_high[hn])
        bins = bins_l + bins_h
    else:
        bins, _ = _pack_bins(NB, np.arange(N), deg, np.zeros(N))
        is_low = np.ones(N, dtype=bool)

    # permutation: pos[old] = new index; bins padded with virtual rows
    pos = np.full(N, -1, dtype=np.int64)
    bin_nodes = np.full((NB, 128), -1, dtype=np.int64)
    for b, nodes in enumerate(bins):
        nodes = np.asarray(nodes, dtype=np.int64)
        bin_nodes[b, :len(nodes)] = nodes
        pos[nodes] = b * 128 + np.arange(len(nodes))

    # per-edge: bin, dst_local, low/high, gather idx
    e_pos_dst = pos[dst]
    e_bin = e_pos_dst // 128
    e_row = e_pos_dst % 128
    e_pos_src = pos[src]
    e_is_low = e_pos_src < lowcap

    # group edges per (bin, range)
    # sort by (bin, is_low desc) so lows first
    order = np.lexsort((~e_is_low, e_bin))
    sb, srow, sidx, slow = e_bin[order], e_row[order], e_pos_src[order], e_is_low[order]

    nl = np.zeros(NB, dtype=np.int64)
    nh = np.zeros(NB, dtype=np.int64)
    np.add.at(nl, e_bin[e_is_low], 1)
    np.add.at(nh, e_bin[~e_is_low], 1)
    # virtual rows get a self-edge in the low group
    nvirt = (bin_nodes < 0).sum(axis=1)
    TL = int(np.ceil((nl + nvirt).max() / 128.0))
    TH = int(np.ceil(nh.max() / 128.0)) if NPAD > lowcap else 0
    TPB = TL + TH
    low_calls = _chunks(TL)
    high_calls = _chunks(TH)
    calls_per_bin = [(c, 0) for c in low_calls] + [(c, 1) for c in high_calls]

    # slot arrays per bin: idx (int16), dcol (float), U one-hot
    slot_idx = np.zeros((NB, TPB * 128), dtype=np.int64)
    slot_dcol = np.full((NB, TPB * 128), DUMMY_DCOL, dtype=np.float64)
    # fill real edges
    # low edges occupy slots [0, nl), high edges [TL*128, TL*128+nh)
    starts = np.searchsorted(sb, np.arange(NB))
    ends = np.searchsorted(sb, np.arange(NB), side="right")
    for b in range(NB):
        s, e = starts[b], ends[b]
        lo_cnt = int(np.count_nonzero(slow[s:e]))
        hi_cnt = (e - s) - lo_cnt
        assert lo_cnt <= TL * 128 - nvirt[b] and hi_cnt <= TH * 128
        slot_idx[b, :lo_cnt] = sidx[s:s + lo_cnt]
        slot_dcol[b, :lo_cnt] = srow[s:s + lo_cnt]
        # virtual self-edges right after real low edges
        nv = int(nvirt[b])
        if nv:
            vrows = np.flatnonzero(bin_nodes[b] < 0)
            slot_idx[b, lo_cnt:lo_cnt + nv] = 0
            slot_dcol[b, lo_cnt:lo_cnt + nv] = vrows
        if hi_cnt:
            slot_idx[b, TL * 128:TL * 128 + hi_cnt] = sidx[s + lo_cnt:e] - hibase
            slot_dcol[b, TL * 128:TL * 128 + hi_cnt] = srow[s + lo_cnt:e]
    assert slot_idx.min() >= 0 and slot_idx.max() < 32768

    # wrapped int16 gather indices per call, replicated to 8 groups of 16
    def wrap(idx_flat):
        ni = len(idx_flat)
        a = idx_flat.astype(np.int16).reshape(ni // 16, 16).T  # [16, ni/16]
        w = np.tile(a, (8, 1))  # [128, ni/16]
        return w

    core_gidx = []
    core_dcol = []
    core_U = []
    core_x = []
    NBPC = NB // n_cores
    for c in range(n_cores):
        gx = []
        for b in range(c * NBPC, (c + 1) * NBPC):
            off = 0
            for ntile, rng in calls_per_bin:
                lo = (0 if rng == 0 else TL * 128) + (off if rng == 0 else off - TL * 128)
                gx.append(wrap(slot_idx[b, off:off + ntile * 128]))
                off += ntile * 128
            assert off == TPB * 128
        core_gidx.append(np.concatenate(gx, axis=1))
        dc = slot_dcol[c * NBPC:(c + 1) * NBPC].reshape(NBPC * TPB, 128)
        # dcol layout [128 partitions(e), NBPC*TPB]
        core_dcol.append(
            np.ascontiguousarray(dc.T).astype(ml_dtypes.bfloat16))
        # U one-hot [128 i, tiles*128 e]
        dloc = slot_dcol[c * NBPC:(c + 1) * NBPC].reshape(-1)  # [NBPC*TPB*128]
        U = (dloc[None, :] == np.arange(128)[:, None]).astype(ml_dtypes.bfloat16)
        core_U.append(np.ascontiguousarray(U))
        # x shard in permuted order (virtual rows zero)
        xs = np.zeros((NBPC * 128, D), dtype=np.float32)
        nodes = bin_nodes[c * NBPC:(c + 1) * NBPC].reshape(-1)
        valid = nodes >= 0
        xs[valid] = np.asarray(x)[nodes[valid]]
        core_x.append(xs)

    p = Plan()
    p.N, p.E, p.NB, p.NPAD, p.NBPC = N, E, NB, NPAD, NBPC
    p.TL, p.TH, p.TPB = TL, TH, TPB
    p.calls_per_bin = calls_per_bin
    p.lowcap, p.hibase = lowcap, hibase
    p.n_cores = n_cores
    p.core_gidx, p.core_dcol, p.core_U, p.core_x = (
        core_gidx, core_dcol, core_U, core_x)
    p.bin_nodes = bin_nodes
    p.pos = pos
    p.idx_cols = core_gidx[0].shape[1]
    return p


def host_weights(Wl, bl, Wr, br, att, bias, L):
    """Shared (replicated) weight tensors."""
    w = {}
    w["wl"] = np.ascontiguousarray(
        np.concatenate([np.asarray(Wl[l], np.float32) for l in range(L)], axis=1)
    ).astype(ml_dtypes.bfloat16)  # [128, L*128]
    w["wr"] = np.ascontiguousarray(
        np.concatenate([np.asarray(Wr[l], np.float32) for l in range(L)], axis=1)
    ).astype(ml_dtypes.bfloat16)
    w["att"] = np.ascontiguousarray(
        np.stack([np.asarray(att[l, 0], np.float32) for l in range(L)], axis=1)
    ).astype(ml_dtypes.bfloat16)  # [128, L]
    w["att2"] = np.ascontiguousarray(
        np.stack([np.asarray(-0.2 * att[l, 0], np.float32) for l in range(L)],
                 axis=1)).astype(ml_dtypes.bfloat16)  # [128, L]
    # lrelu bias = bl + br per layer, [128, L] f32
    w["lrb"] = np.ascontiguousarray(
        np.stack([np.asarray(bl[l] + br[l], np.float32) for l in range(L)], axis=1))
    w["lrbn"] = np.ascontiguousarray(-w["lrb"])
    # epilogue bias = bl + bias per layer, replicated [128, L*128] f32
    be = np.concatenate(
        [np.tile((np.asarray(bl[l] + bias[l], np.float32))[None, :], (128, 1))
         for l in range(L)], axis=1)
    w["be"] = np.ascontiguousarray(be)
    ident = np.eye(128, dtype=np.float32)
    w["idf"] = ident
    w["idb"] = ident.astype(ml_dtypes.bfloat16)
    w["iota"] = np.tile(np.arange(128, dtype=np.float32)[None, :], (128, 4)
                        ).astype(ml_dtypes.bfloat16)  # [128, 4*128]
    return w


# ----------------------------------------------------------------------------
# Device program
# ----------------------------------------------------------------------------

def build_nc(p, L=3):
    NBPC, TL, TH, TPB = p.NBPC, p.TL, p.TH, p.TPB
    NSH = NBPC * 128          # shard rows
    n_cores = p.n_cores
    NPAD = p.NPAD

    nc = bacc.Bacc("TRN2", target_bir_lowering=False, debug=False,
                   num_devices=n_cores, num_swdge_queues=4)

    # --- I/O ---
    x_in = nc.dram_tensor("x_shard", [NSH, D], F32, kind="ExternalInput")
    gidx_in = nc.dram_tensor("gidx", [128, p.idx_cols], I16, kind="ExternalInput")
    dcol_in = nc.dram_tensor("dcol", [128, NBPC * TPB], BF16, kind="ExternalInput")
    U_in = nc.dram_tensor("umat", [128, NBPC * TPB * 128], BF16,
                          kind="ExternalInput")
    wl_in = nc.dram_tensor("wl", [128, L * 128], BF16, kind="ExternalInput")
    wr_in = nc.dram_tensor("wr", [128, L * 128], BF16, kind="ExternalInput")
    att_in = nc.dram_tensor("att", [128, L], BF16, kind="ExternalInput")
    att2_in = nc.dram_tensor("att2", [128, L], BF16, kind="ExternalInput")
    lrb_in = nc.dram_tensor("lrb", [128, L], F32, kind="ExternalInput")
    lrbn_in = nc.dram_tensor("lrbn", [128, L], F32, kind="ExternalInput")
    be_in = nc.dram_tensor("be", [128, L * 128], F32, kind="ExternalInput")
    idf_in = nc.dram_tensor("idf", [128, 128], F32, kind="ExternalInput")
    idb_in = nc.dram_tensor("idb", [128, 128], BF16, kind="ExternalInput")
    iota_in = nc.dram_tensor("iota", [128, 4 * 128], BF16, kind="ExternalInput")
    out_t = nc.dram_tensor("out", [NSH, D], F32, kind="ExternalOutput")

    with tile.TileContext(nc) as tc:
        with (
            tc.tile_pool(name="const", bufs=1) as constp,
            tc.tile_pool(name="sb", bufs=2) as sb,
            tc.tile_pool(name="sb3", bufs=6) as sb3,
            tc.tile_pool(name="psum", bufs=2, space="PSUM") as ps,
            tc.tile_pool(name="dram", bufs=1, space="DRAM") as dram,
        ):
            # ---- constants / resident tensors ----
            gidx = constp.tile([128, p.idx_cols], I16)
            nc.sync.dma_start(gidx[:], gidx_in[:])
            dcol = constp.tile([128, NBPC * TPB], BF16)
            nc.sync.dma_start(dcol[:], dcol_in[:])
            wl = constp.tile([128, L * 128], BF16)
            nc.sync.dma_start(wl[:], wl_in[:])
            wr = constp.tile([128, L * 128], BF16)
            nc.sync.dma_start(wr[:], wr_in[:])
            att = constp.tile([128, L], BF16)
            nc.sync.dma_start(att[:], att_in[:])
            att2 = constp.tile([128, L], BF16)
            nc.sync.dma_start(att2[:], att2_in[:])
            lrb = constp.tile([128, L], F32)
            nc.sync.dma_start(lrb[:], lrb_in[:])
            lrbn = constp.tile([128, L], F32)
            nc.sync.dma_start(lrbn[:], lrbn_in[:])
            be = constp.tile([128, L * 128], F32)
            nc.sync.dma_start(be[:], be_in[:])
            idf = constp.tile([128, 128], F32)
            nc.sync.dma_start(idf[:], idf_in[:])
            idb = constp.tile([128, 128], BF16)
            nc.sync.dma_start(idb[:], idb_in[:])
            iota = constp.tile([128, 4 * 128], BF16)
            nc.sync.dma_start(iota[:], iota_in[:])
            # xr kept SBUF-resident per layer
            xr_res = constp.tile([128, NBPC * 128], BF16)

            # ---- DRAM scratch ----
            h_cur = dram.tile([NSH, D], F32)       # current layer input
            xl_shard = dram.tile([NSH, D], BF16)
            xl_full = dram.tile([NPAD, D], BF16)
            nc.sync.dma_start(h_cur[:], x_in[:])

            for l in range(L):
                # ======== Phase A: xl/xr shard + AllGather ========
                for t in range(NBPC):
                    h_t = sb.tile([128, 128], F32, tag="ht")
                    nc.sync.dma_start(h_t[:], h_cur[t * 128:(t + 1) * 128, :])
                    hT_ps = ps.tile([128, 128], F32, tag="wT")
                    nc.tensor.matmul(hT_ps[:], lhsT=h_t[:], rhs=idf[:],
                                     start=True, stop=True)
                    hT = sb.tile([128, 128], BF16, tag="hT")
                    nc.scalar.copy(hT[:], hT_ps[:])
                    xl_ps = ps.tile([128, 128], F32, tag="ag")
                    nc.tensor.matmul(xl_ps[:], lhsT=hT[:],
                                     rhs=wl[:, l * 128:(l + 1) * 128],
                                     start=True, stop=True)
                    xl_t = sb.tile([128, 128], BF16, tag="xlt")
                    nc.scalar.copy(xl_t[:], xl_ps[:])
                    nc.sync.dma_start(xl_shard[t * 128:(t + 1) * 128, :], xl_t[:])
                    xr_ps = ps.tile([128, 128], F32, tag="sc")
                    nc.tensor.matmul(xr_ps[:], lhsT=hT[:],
                                     rhs=wr[:, l * 128:(l + 1) * 128],
                                     start=True, stop=True)
                    nc.scalar.copy(xr_res[:, t * 128:(t + 1) * 128], xr_ps[:])

                nc.gpsimd.collective_compute(
                    "AllGather",
                    mybir.AluOpType.bypass,
                    replica_groups=[list(range(n_cores))],
                    ins=[xl_shard.opt()],
                    outs=[xl_full.opt()],
                )

                # ======== Phase B: per-bin edge processing ========
                idx_off = 0
                for b in range(NBPC):
                    u_bin = sb3.tile([128, TPB, 128], BF16, tag="u")
                    # gather calls
                    tile_off = 0
                    for ci, (ntile, rng) in enumerate(p.calls_per_bin):
                        ni = ntile * 128
                        base = 0 if rng == 0 else p.hibase
                        src_view = xl_full[base:base + p.lowcap, :]
                        nc.gpsimd.dma_gather(
                            out_ap=u_bin[:, tile_off:tile_off + ntile, :],
                            in_ap=src_view,
                            idxs_ap=gidx[:, idx_off:idx_off + ni // 16],
                            num_idxs=ni,
                            num_idxs_reg=ni,
                            elem_size=D,
                            queue_num=(b * len(p.calls_per_bin) + ci) % 4,
                            single_packet=False,
                        )
                        idx_off += ni // 16
                        tile_off += ntile
                    # U for this bin
                    U_bin = sb3.tile([128, TPB * 128], BF16, tag="U")
                    nc.sync.dma_start(
                        U_bin[:],
                        U_in[:, (b * TPB) * 128:((b + 1) * TPB) * 128])

                    xr_tile = xr_res[:, b * 128:(b + 1) * 128]
                    da_ps = ps.tile([128, 129], F32, tag="da")

                    # batches of up to 4 tiles
                    bk_sizes = _chunks(TPB, 4)
                    t0 = 0
                    for bk in bk_sizes:
                        wT_ps = ps.tile([128, 4 * 128], F32, tag="wT")
                        # one wide xr@U matmul seeds the chunk (1 LDW), then
                        # per-tile transposes accumulate on top
                        nc.tensor.matmul(
                            wT_ps[:, :bk * 128],
                            lhsT=xr_tile,
                            rhs=U_bin[:, t0 * 128:(t0 + bk) * 128],
                            start=True, stop=False)
                        for j in range(bk):
                            t = t0 + j
                            nc.tensor.matmul(
                                wT_ps[:, j * 128:(j + 1) * 128],
                                lhsT=u_bin[:, t, :], rhs=idb[:],
                                start=False, stop=True)
                        lre = sb.tile([128, 4 * 128], BF16, tag="lre")
                        nc.scalar.activation(
                            lre[:, :bk * 128], wT_ps[:, :bk * 128],
                            mybir.ActivationFunctionType.Relu,
                            bias=lrb[:, l:l + 1], scale=1.0)
                        lrn = sb.tile([128, 4 * 128], BF16, tag="lrn")
                        nc.scalar.activation(
                            lrn[:, :bk * 128], wT_ps[:, :bk * 128],
                            mybir.ActivationFunctionType.Relu,
                            bias=lrbn[:, l:l + 1], scale=-1.0)
                        sc_ps = ps.tile([128, 4], F32, tag="sc")
                        for j in range(bk):
                            nc.tensor.matmul(
                                sc_ps[:, j:j + 1],
                                lhsT=lre[:, j * 128:(j + 1) * 128],
                                rhs=att[:, l:l + 1], start=True, stop=False)
                            nc.tensor.matmul(
                                sc_ps[:, j:j + 1],
                                lhsT=lrn[:, j * 128:(j + 1) * 128],
                                rhs=att2[:, l:l + 1], start=False, stop=True)
                        # pp4 per tile: [pad, p8, pu(128)] so denom+agg is
                        # one matmul streaming cols 1:130
                        pp4 = sb.tile([128, 4, 130], BF16, tag="pp4")
                        nc.scalar.activation(
                            pp4[:, :bk, 1:2],
                            sc_ps[:, :bk].rearrange("p (b o) -> p b o", o=1),
                            mybir.ActivationFunctionType.Exp)
                        A8 = sb.tile([128, 4, 128], BF16, tag="A8")
                        nc.vector.tensor_tensor(
                            out=A8[:, :bk, :],
                            in0=dcol[:, b * TPB + t0:b * TPB + t0 + bk]
                                .to_broadcast([128, bk, 128]),
                            in1=iota[:, :bk * 128].rearrange(
                                "p (b e) -> p b e", e=128),
                            op=mybir.AluOpType.is_equal)
                        nc.vector.tensor_tensor(
                            out=pp4[:, :bk, 2:130],
                            in0=u_bin[:, t0:t0 + bk, :],
                            in1=pp4[:, :bk, 1:2].to_broadcast([128, bk, 128]),
                            op=mybir.AluOpType.mult)
                        for j in range(bk):
                            t = t0 + j
                            nc.tensor.matmul(
                                da_ps[:], lhsT=A8[:, j, :],
                                rhs=pp4[:, j, 1:130],
                                start=(t == 0), stop=(t == TPB - 1))
                        t0 += bk

                    # ---- epilogue ----
                    rec = sb.tile([128, 1], F32, tag="rec")
                    nc.vector.reciprocal(rec[:], da_ps[:, 0:1])
                    hp = sb.tile([128, 128], F32, tag="hp")
                    nc.vector.tensor_scalar_mul(hp[:], da_ps[:, 1:129], rec[:])
                    hb = sb.tile([128, 128], F32, tag="hb")
                    nc.vector.tensor_add(
                        hb[:], hp[:], be[:, l * 128:(l + 1) * 128])
                    if l < L - 1:
                        hn = sb.tile([128, 128], F32, tag="hn")
                        nc.scalar.activation(
                            hn[:], hb[:], mybir.ActivationFunctionType.Relu)
                        nc.sync.dma_start(
                            h_cur[b * 128:(b + 1) * 128, :], hn[:])
                    else:
                        z = sb.tile([128, 128], F32, tag="hn")
                        nc.scalar.activation(
                            z[:], hb[:], mybir.ActivationFunctionType.Relu)
                        m = sb.tile([128, 1], F32, tag="m")
                        nc.vector.tensor_reduce(
                            m[:], z[:], axis=mybir.AxisListType.X,
                            op=mybir.AluOpType.max)
                        mn = sb.tile([128, 1], F32, tag="mn")
                        nc.scalar.mul(mn[:], m[:], -1.0)
                        zc = sb.tile([128, 128], F32, tag="zc")
                        nc.scalar.activation(
                            zc[:], z[:],
                            mybir.ActivationFunctionType.Identity,
                            bias=mn[:], scale=1.0)
                        ez = sb.tile([128, 128], F32, tag="ez")
                        nc.scalar.activation(
                            ez[:], zc[:], mybir.ActivationFunctionType.Exp)
                        s = sb.tile([128, 1], F32, tag="s")
                        nc.vector.tensor_reduce(
                            s[:], ez[:], axis=mybir.AxisListType.X,
                            op=mybir.AluOpType.add)
                        ls = sb.tile([128, 1], F32, tag="ls")
                        nc.scalar.activation(
                            ls[:], s[:], mybir.ActivationFunctionType.Ln)
                        lsn = sb.tile([128, 1], F32, tag="lsn")
                        nc.scalar.mul(lsn[:], ls[:], -1.0)
                        o = sb.tile([128, 128], F32, tag="o")
                        nc.scalar.activation(
                            o[:], zc[:],
                            mybir.ActivationFunctionType.Identity,
                            bias=lsn[:], scale=1.0)
                        nc.sync.dma_start(
                            out_t[b * 128:(b + 1) * 128, :], o[:])

    nc.compile()
    return nc


# ----------------------------------------------------------------------------
# Entry point
# ----------------------------------------------------------------------------

def kernel(x, edge_index, Wl, bl, Wr, br, att, bias):
    n_cores = 8
    _sim = False
    x = np.asarray(x)
    edge_index = np.asarray(edge_index)
    L = np.asarray(Wl).shape[0]
    p = host_prep(x, edge_index, n_cores)
    w = host_weights(np.asarray(Wl), np.asarray(bl), np.asarray(Wr),
                     np.asarray(br), np.asarray(att), np.asarray(bias), L)
    nc = build_nc(p, L=L)

    in_maps = []
    for c in range(n_cores):
        m = {
            "x_shard": p.core_x[c],
            "gidx": p.core_gidx[c],
            "dcol": p.core_dcol[c],
            "umat": p.core_U[c],
            "wl": w["wl"], "wr": w["wr"], "att": w["att"],
            "att2": w["att2"], "lrb": w["lrb"], "lrbn": w["lrbn"],
            "be": w["be"],
            "idf": w["idf"], "idb": w["idb"], "iota": w["iota"],
        }
        in_maps.append(m)

    if _sim:
        from concourse.bass_interp import CoreSim, MultiCoreSim
        if n_cores == 1:
            sims = [CoreSim(nc)]
            for k, v in in_maps[0].items():
                sims[0].tensor(k)[:] = v
            sims[0].simulate()
            shard_outs = [sims[0].tensor("out").copy()]
        else:
            msim = MultiCoreSim(nc, num_cores=n_cores, trace=False)
            for c, core in sorted(msim.cores.items()):
                for k, v in in_maps[c].items():
                    core.tensor(k)[:] = v
            msim.simulate()
            shard_outs = [msim.cores[c].tensor("out").copy()
                          for c in range(n_cores)]
        res = None
    else:
        res = run_bass_kernel_spmd(nc, in_maps, core_ids=list(range(n_cores)))
        shard_outs = [res.results[c]["out"] for c in range(n_cores)]

    # un-permute
    out = np.zeros((p.N, D), dtype=np.float32)
    for c in range(n_cores):
        nodes = p.bin_nodes[c * p.NBPC:(c + 1) * p.NBPC].reshape(-1)
        valid = nodes >= 0
        out[nodes[valid]] = shard_outs[c][valid]
    kernel.last_results = res
    return out



# revision 14
# speedup vs baseline: 1.3578x; 1.0013x over previous
"""GATv2 (3-layer, H=1, D=128) on 8 Trainium2 NeuronCores via Bass/Tile.

Self-contained: host preprocessing (bin packing, edge tiling, int16 gather
indices), bass program builder, and the `kernel(**inputs)` entry point.

Strategy (dst-node sharding, uniform SPMD program):
  - Permute nodes into bins of 128 dst rows, 2-D balanced by (low,high)
    in-degree so every bin has exactly TL low + TH high edge tiles of 128.
  - Per layer: each core computes xl/xr for its shard (bf16), AllGather of
    xl -> xl_full in every core's HBM.
  - Per bin: dma_gather source features u[e,d] (bf16, 256B rows); per edge
    tile: PE transpose u -> wT psum, += xr^T-onehot matmul (U streamed from
    HBM); ACT LeakyRelu(+bl+br bias); score = lreluT^T @ att (PE);
    exp (ACT); denom/agg via one-hot matmuls accumulated in PSUM; epilogue
    recip + bias + relu (+ log_softmax on the last layer).
"""
import numpy as np
import ml_dtypes

import concourse.bacc as bacc
import concourse.mybir as mybir
import concourse.tile as tile
from concourse.bass_utils import run_bass_kernel_spmd

F32 = mybir.dt.float32
BF16 = mybir.dt.bfloat16
I16 = mybir.dt.int16
SLOPE = 0.2
D = 128
LOWCAP = 32768
DUMMY_DCOL = 200.0  # never matches iota 0..127 -> zero one-hot column


# ----------------------------------------------------------------------------
# Host preprocessing
# ----------------------------------------------------------------------------

def _pack_bins(n_bins, node_ids, w_low, w_high, cap=128,
               cap_low=np.inf, cap_high=np.inf):
    order = np.argsort(-(w_low + w_high), kind="stable")
    bins = [[] for _ in range(n_bins)]
    sums = np.zeros((n_bins, 2))
    counts = np.zeros(n_bins, dtype=np.int64)
    tgt_l = max(w_low.sum() / n_bins, 1.0)
    tgt_h = max(w_high.sum() / n_bins, 1.0)
    for idx in order:
        load = (sums[:, 0] + w_low[idx]) / tgt_l + (sums[:, 1] + w_high[idx]) / tgt_h
        load = np.where(counts >= cap, np.inf, load)
        load = np.where(sums[:, 0] + w_low[idx] > cap_low, np.inf, load)
        load = np.where(sums[:, 1] + w_high[idx] > cap_high, np.inf, load)
        if not np.isfinite(load).any():
            return None, None
        b = int(np.argmin(load))
        bins[b].append(node_ids[idx])
        sums[b, 0] += w_low[idx]
        sums[b, 1] += w_high[idx]
        counts[b] += 1
    return bins, sums


def _pack_bins_capped(n_bins, node_ids, w_low, w_high, cap=128):
    """Try progressively looser (cap_low, cap_high) slot caps so the max
    bin needs fewer 128-edge tiles; fall back to uncapped."""
    for cl, ch in ((1530, 764), (1530, 890), (1658, 890),
                   (np.inf, np.inf)):
        bins, sums = _pack_bins(n_bins, node_ids, w_low, w_high, cap=cap,
                                cap_low=cl, cap_high=ch)
        if bins is not None:
            return bins, sums
    raise AssertionError("unreachable")


def _chunks(n, cap=12):
    out = []
    while n > 0:
        c = min(n, cap)
        out.append(c)
        n -= c
    return out


class Plan:
    pass


def host_prep(x, edge_index, n_cores):
    """Returns Plan with per-core input tensors + structural constants."""
    N = x.shape[0]
    E = edge_index.shape[1]
    src = np.concatenate([np.asarray(edge_index[0], dtype=np.int64),
                          np.arange(N, dtype=np.int64)])
    dst = np.concatenate([np.asarray(edge_index[1], dtype=np.int64),
                          np.arange(N, dtype=np.int64)])

    NB = int(np.ceil(N / 128.0))
    NB = int(np.ceil(NB / n_cores)) * n_cores
    NPAD = NB * 128
    assert NPAD <= 65536
    lowcap = min(LOWCAP, NPAD)  # small configs: everything "low"
    hibase = NPAD - lowcap

    deg = np.bincount(dst, minlength=N).astype(np.float64)
    if NPAD > lowcap:
        # pass 1: balance total degree; defines the low/high node split
        bins1, _ = _pack_bins(NB, np.arange(N), deg, np.zeros(N))
        n_low_bins = lowcap // 128
        low_nodes = np.concatenate(
            [np.asarray(bins1[i], dtype=np.int64) for i in range(n_low_bins)]
        ) if n_low_bins else np.zeros(0, dtype=np.int64)
        is_low = np.zeros(N, dtype=bool)
        is_low[low_nodes] = True
        deg_low = np.bincount(dst[is_low[src]], minlength=N).astype(np.float64)
        deg_high = deg - deg_low
        ln = np.flatnonzero(is_low)
        hn = np.flatnonzero(~is_low)
        bins_l, _ = _pack_bins_capped(n_low_bins, ln, deg_low[ln],
                                      deg_high[ln])
        bins_h, _ = _pack_bins_capped(NB - n_low_bins, hn, deg_low[hn],
                                      deg_high[hn])
        bins = bins_l + bins_h
    else:
        bins, _ = _pack_bins(NB, np.arange(N), deg, np.zeros(N))
        is_low = np.ones(N, dtype=bool)

    # permutation: pos[old] = new index; bins padded with virtual rows
    pos = np.full(N, -1, dtype=np.int64)
    bin_nodes = np.full((NB, 128), -1, dtype=np.int64)
    for b, nodes in enumerate(bins):
        nodes = np.asarray(nodes, dtype=np.int64)
        bin_nodes[b, :len(nodes)] = nodes
        pos[nodes] = b * 128 + np.arange(len(nodes))

    # per-edge: bin, dst_local, low/high, gather idx
    e_pos_dst = pos[dst]
    e_bin = e_pos_dst // 128
    e_row = e_pos_dst % 128
    e_pos_src = pos[src]
    e_is_low = e_pos_src < lowcap

    # group edges per (bin, range)
    # sort by (bin, is_low desc) so lows first
    order = np.lexsort((~e_is_low, e_bin))
    sb, srow, sidx, slow = e_bin[order], e_row[order], e_pos_src[order], e_is_low[order]

    nl = np.zeros(NB, dtype=np.int64)
    nh = np.zeros(NB, dtype=np.int64)
    np.add.at(nl, e_bin[e_is_low], 1)
    np.add.at(nh, e_bin[~e_is_low], 1)
    # virtual rows get a self-edge in the low group
    nvirt = (bin_nodes < 0).sum(axis=1)
    TL = int(np.ceil((nl + nvirt).max() / 128.0))
    TH = int(np.ceil(nh.max() / 128.0)) if NPAD > lowcap else 0
    TPB = TL + TH
    low_calls = _chunks(TL)
    high_calls = _chunks(TH)
    calls_per_bin = [(c, 0) for c in low_calls] + [(c, 1) for c in high_calls]

    # slot arrays per bin: idx (int16), dcol (float), U one-hot
    slot_idx = np.zeros((NB, TPB * 128), dtype=np.int64)
    slot_dcol = np.full((NB, TPB * 128), DUMMY_DCOL, dtype=np.float64)
    # fill real edges
    # low edges occupy slots [0, nl), high edges [TL*128, TL*128+nh)
    starts = np.searchsorted(sb, np.arange(NB))
    ends = np.searchsorted(sb, np.arange(NB), side="right")
    for b in range(NB):
        s, e = starts[b], ends[b]
        lo_cnt = int(np.count_nonzero(slow[s:e]))
        hi_cnt = (e - s) - lo_cnt
        assert lo_cnt <= TL * 128 - nvirt[b] and hi_cnt <= TH * 128
        slot_idx[b, :lo_cnt] = sidx[s:s + lo_cnt]
        slot_dcol[b, :lo_cnt] = srow[s:s + lo_cnt]
        # virtual self-edges right after real low edges
        nv = int(nvirt[b])
        if nv:
            vrows = np.flatnonzero(bin_nodes[b] < 0)
            slot_idx[b, lo_cnt:lo_cnt + nv] = 0
            slot_dcol[b, lo_cnt:lo_cnt + nv] = vrows
        if hi_cnt:
            slot_idx[b, TL * 128:TL * 128 + hi_cnt] = sidx[s + lo_cnt:e] - hibase
            slot_dcol[b, TL * 128:TL * 128 + hi_cnt] = srow[s + lo_cnt:e]
    assert slot_idx.min() >= 0 and slot_idx.max() < 32768

    # wrapped int16 gather indices per call, replicated to 8 groups of 16
    def wrap(idx_flat):
        ni = len(idx_flat)
        a = idx_flat.astype(np.int16).reshape(ni // 16, 16).T  # [16, ni/16]
        w = np.tile(a, (8, 1))  # [128, ni/16]
        return w

    core_gidx = []
    core_dcol = []
    core_U = []
    core_x = []
    NBPC = NB // n_cores
    for c in range(n_cores):
        gx = []
        for b in range(c * NBPC, (c + 1) * NBPC):
            off = 0
            for ntile, rng in calls_per_bin:
                lo = (0 if rng == 0 else TL * 128) + (off if rng == 0 else off - TL * 128)
                gx.append(wrap(slot_idx[b, off:off + ntile * 128]))
                off += ntile * 128
            assert off == TPB * 128
        core_gidx.append(np.concatenate(gx, axis=1))
        dc = slot_dcol[c * NBPC:(c + 1) * NBPC].reshape(NBPC * TPB, 128)
        # dcol layout [128 partitions(e), NBPC*TPB]
        core_dcol.append(
            np.ascontiguousarray(dc.T).astype(ml_dtypes.bfloat16))
        # U one-hot [128 i, tiles*128 e]
        dloc = slot_dcol[c * NBPC:(c + 1) * NBPC].reshape(-1)  # [NBPC*TPB*128]
        U = (dloc[None, :] == np.arange(128)[:, None]).astype(ml_dtypes.bfloat16)
        core_U.append(np.ascontiguousarray(U))
        # x shard in permuted order (virtual rows zero)
        xs = np.zeros((NBPC * 128, D), dtype=np.float32)
        nodes = bin_nodes[c * NBPC:(c + 1) * NBPC].reshape(-1)
        valid = nodes >= 0
        xs[valid] = np.asarray(x)[nodes[valid]]
        core_x.append(xs)

    p = Plan()
    p.N, p.E, p.NB, p.NPAD, p.NBPC = N, E, NB, NPAD, NBPC
    p.TL, p.TH, p.TPB = TL, TH, TPB
    p.calls_per_bin = calls_per_bin
    p.lowcap, p.hibase = lowcap, hibase
    p.n_cores = n_cores
    p.core_gidx, p.core_dcol, p.core_U, p.core_x = (
        core_gidx, core_dcol, core_U, core_x)
    p.bin_nodes = bin_nodes
    p.pos = pos
    p.idx_cols = core_gidx[0].shape[1]
    return p


def host_weights(Wl, bl, Wr, br, att, bias, L):
    """Shared (replicated) weight tensors."""
    w = {}
    w["wl"] = np.ascontiguousarray(
        np.concatenate([np.asarray(Wl[l], np.float32) for l in range(L)], axis=1)
    ).astype(ml_dtypes.bfloat16)  # [128, L*128]
    w["wr"] = np.ascontiguousarray(
        np.concatenate([np.asarray(Wr[l], np.float32) for l in range(L)], axis=1)
    ).astype(ml_dtypes.bfloat16)
    w["att"] = np.ascontiguousarray(
        np.stack([np.asarray(att[l, 0], np.float32) for l in range(L)], axis=1)
    ).astype(ml_dtypes.bfloat16)  # [128, L]
    w["att2"] = np.ascontiguousarray(
        np.stack([np.asarray(-0.2 * att[l, 0], np.float32) for l in range(L)],
                 axis=1)).astype(ml_dtypes.bfloat16)  # [128, L]
    # lrelu bias = bl + br per layer, [128, L] f32
    w["lrb"] = np.ascontiguousarray(
        np.stack([np.asarray(bl[l] + br[l], np.float32) for l in range(L)], axis=1))
    w["lrbn"] = np.ascontiguousarray(-w["lrb"])
    # epilogue bias = bl + bias per layer, replicated [128, L*128] f32
    be = np.concatenate(
        [np.tile((np.asarray(bl[l] + bias[l], np.float32))[None, :], (128, 1))
         for l in range(L)], axis=1)
    w["be"] = np.ascontiguousarray(be)
    ident = np.eye(128, dtype=np.float32)
    w["idf"] = ident
    w["idb"] = ident.astype(ml_dtypes.bfloat16)
    w["iota"] = np.tile(np.arange(128, dtype=np.float32)[None, :], (128, 4)
                        ).astype(ml_dtypes.bfloat16)  # [128, 4*128]
    return w


# ----------------------------------------------------------------------------
# Device program
# ----------------------------------------------------------------------------

def build_nc(p, L=3):
    NBPC, TL, TH, TPB = p.NBPC, p.TL, p.TH, p.TPB
    NSH = NBPC * 128          # shard rows
    n_cores = p.n_cores
    NPAD = p.NPAD

    nc = bacc.Bacc("TRN2", target_bir_lowering=False, debug=False,
                   num_devices=n_cores, num_swdge_queues=4)

    # --- I/O ---
    x_in = nc.dram_tensor("x_shard", [NSH, D], F32, kind="ExternalInput")
    gidx_in = nc.dram_tensor("gidx", [128, p.idx_cols], I16, kind="ExternalInput")
    dcol_in = nc.dram_tensor("dcol", [128, NBPC * TPB], BF16, kind="ExternalInput")
    U_in = nc.dram_tensor("umat", [128, NBPC * TPB * 128], BF16,
                          kind="ExternalInput")
    wl_in = nc.dram_tensor("wl", [128, L * 128], BF16, kind="ExternalInput")
    wr_in = nc.dram_tensor("wr", [128, L * 128], BF16, kind="ExternalInput")
    att_in = nc.dram_tensor("att", [128, L], BF16, kind="ExternalInput")
    att2_in = nc.dram_tensor("att2", [128, L], BF16, kind="ExternalInput")
    lrb_in = nc.dram_tensor("lrb", [128, L], F32, kind="ExternalInput")
    lrbn_in = nc.dram_tensor("lrbn", [128, L], F32, kind="ExternalInput")
    be_in = nc.dram_tensor("be", [128, L * 128], F32, kind="ExternalInput")
    idf_in = nc.dram_tensor("idf", [128, 128], F32, kind="ExternalInput")
    idb_in = nc.dram_tensor("idb", [128, 128], BF16, kind="ExternalInput")
    iota_in = nc.dram_tensor("iota", [128, 4 * 128], BF16, kind="ExternalInput")
    out_t = nc.dram_tensor("out", [NSH, D], F32, kind="ExternalOutput")

    with tile.TileContext(nc) as tc:
        with (
            tc.tile_pool(name="const", bufs=1) as constp,
            tc.tile_pool(name="sb", bufs=2) as sb,
            tc.tile_pool(name="sb3", bufs=6) as sb3,
            tc.tile_pool(name="psum", bufs=2, space="PSUM") as ps,
            tc.tile_pool(name="dram", bufs=1, space="DRAM") as dram,
        ):
            # ---- constants / resident tensors ----
            gidx = constp.tile([128, p.idx_cols], I16)
            nc.sync.dma_start(gidx[:], gidx_in[:])
            dcol = constp.tile([128, NBPC * TPB], BF16)
            nc.sync.dma_start(dcol[:], dcol_in[:])
            wl = constp.tile([128, L * 128], BF16)
            nc.sync.dma_start(wl[:], wl_in[:])
            wr = constp.tile([128, L * 128], BF16)
            nc.sync.dma_start(wr[:], wr_in[:])
            att = constp.tile([128, L], BF16)
            nc.sync.dma_start(att[:], att_in[:])
            att2 = constp.tile([128, L], BF16)
            nc.sync.dma_start(att2[:], att2_in[:])
            lrb = constp.tile([128, L], F32)
            nc.sync.dma_start(lrb[:], lrb_in[:])
            lrbn = constp.tile([128, L], F32)
            nc.sync.dma_start(lrbn[:], lrbn_in[:])
            be = constp.tile([128, L * 128], F32)
            nc.sync.dma_start(be[:], be_in[:])
            idf = constp.tile([128, 128], F32)
            nc.sync.dma_start(idf[:], idf_in[:])
            idb = constp.tile([128, 128], BF16)
            nc.sync.dma_start(idb[:], idb_in[:])
            iota = constp.tile([128, 4 * 128], BF16)
            nc.sync.dma_start(iota[:], iota_in[:])
            # xr kept SBUF-resident per layer
            xr_res = constp.tile([128, NBPC * 128], BF16)
            # relu(h) of the last layer, held for the deferred log_softmax
            zbuf = constp.tile([128, NBPC * 128], F32)

            # ---- DRAM scratch ----
            h_cur = dram.tile([NSH, D], F32)       # current layer input
            xl_shard = dram.tile([NSH, D], BF16)
            xl_full = dram.tile([NPAD, D], BF16)
            nc.sync.dma_start(h_cur[:], x_in[:])

            for l in range(L):
                # ======== Phase A: xl/xr shard + AllGather ========
                for t in range(NBPC):
                    h_t = sb.tile([128, 128], F32, tag="ht")
                    nc.sync.dma_start(h_t[:], h_cur[t * 128:(t + 1) * 128, :])
                    hT_ps = ps.tile([128, 128], F32, tag="wT")
                    nc.tensor.matmul(hT_ps[:], lhsT=h_t[:], rhs=idf[:],
                                     start=True, stop=True)
                    hT = sb.tile([128, 128], BF16, tag="hT")
                    nc.scalar.copy(hT[:], hT_ps[:])
                    xl_ps = ps.tile([128, 128], F32, tag="ag")
                    nc.tensor.matmul(xl_ps[:], lhsT=hT[:],
                                     rhs=wl[:, l * 128:(l + 1) * 128],
                                     start=True, stop=True)
                    xl_t = sb.tile([128, 128], BF16, tag="xlt")
                    nc.scalar.copy(xl_t[:], xl_ps[:])
                    nc.sync.dma_start(xl_shard[t * 128:(t + 1) * 128, :], xl_t[:])
                    xr_ps = ps.tile([128, 128], F32, tag="sc")
                    nc.tensor.matmul(xr_ps[:], lhsT=hT[:],
                                     rhs=wr[:, l * 128:(l + 1) * 128],
                                     start=True, stop=True)
                    nc.scalar.copy(xr_res[:, t * 128:(t + 1) * 128], xr_ps[:])

                nc.gpsimd.collective_compute(
                    "AllGather",
                    mybir.AluOpType.bypass,
                    replica_groups=[list(range(n_cores))],
                    ins=[xl_shard.opt()],
                    outs=[xl_full.opt()],
                )

                # ======== Phase B: per-bin edge processing ========
                idx_off = 0
                for b in range(NBPC):
                    u_bin = sb3.tile([128, TPB, 128], BF16, tag="u")
                    # gather calls
                    tile_off = 0
                    for ci, (ntile, rng) in enumerate(p.calls_per_bin):
                        ni = ntile * 128
                        base = 0 if rng == 0 else p.hibase
                        src_view = xl_full[base:base + p.lowcap, :]
                        nc.gpsimd.dma_gather(
                            out_ap=u_bin[:, tile_off:tile_off + ntile, :],
                            in_ap=src_view,
                            idxs_ap=gidx[:, idx_off:idx_off + ni // 16],
                            num_idxs=ni,
                            num_idxs_reg=ni,
                            elem_size=D,
                            queue_num=(b * len(p.calls_per_bin) + ci) % 4,
                            single_packet=False,
                        )
                        idx_off += ni // 16
                        tile_off += ntile
                    # U for this bin
                    U_bin = sb3.tile([128, TPB * 128], BF16, tag="U")
                    nc.sync.dma_start(
                        U_bin[:],
                        U_in[:, (b * TPB) * 128:((b + 1) * TPB) * 128])

                    xr_tile = xr_res[:, b * 128:(b + 1) * 128]
                    da_ps = ps.tile([128, 129], F32, tag="da")

                    # batches of up to 4 tiles
                    bk_sizes = _chunks(TPB, 4)
                    t0 = 0
                    for bk in bk_sizes:
                        wT_ps = ps.tile([128, 4 * 128], F32, tag="wT")
                        # one wide xr@U matmul seeds the chunk (1 LDW), then
                        # per-tile transposes accumulate on top
                        nc.tensor.matmul(
                            wT_ps[:, :bk * 128],
                            lhsT=xr_tile,
                            rhs=U_bin[:, t0 * 128:(t0 + bk) * 128],
                            start=True, stop=False)
                        for j in range(bk):
                            t = t0 + j
                            nc.tensor.matmul(
                                wT_ps[:, j * 128:(j + 1) * 128],
                                lhsT=u_bin[:, t, :], rhs=idb[:],
                                start=False, stop=True)
                        lre = sb.tile([128, 4 * 128], BF16, tag="lre")
                        nc.scalar.activation(
                            lre[:, :bk * 128], wT_ps[:, :bk * 128],
                            mybir.ActivationFunctionType.Relu,
                            bias=lrb[:, l:l + 1], scale=1.0)
                        lrn = sb.tile([128, 4 * 128], BF16, tag="lrn")
                        nc.scalar.activation(
                            lrn[:, :bk * 128], wT_ps[:, :bk * 128],
                            mybir.ActivationFunctionType.Relu,
                            bias=lrbn[:, l:l + 1], scale=-1.0)
                        sc_ps = ps.tile([128, 4], F32, tag="sc")
                        for j in range(bk):
                            nc.tensor.matmul(
                                sc_ps[:, j:j + 1],
                                lhsT=lre[:, j * 128:(j + 1) * 128],
                                rhs=att[:, l:l + 1], start=True, stop=False)
                            nc.tensor.matmul(
                                sc_ps[:, j:j + 1],
                                lhsT=lrn[:, j * 128:(j + 1) * 128],
                                rhs=att2[:, l:l + 1], start=False, stop=True)
                        # pp4 per tile: [pad, p8, pu(128)] so denom+agg is
                        # one matmul streaming cols 1:130
                        pp4 = sb.tile([128, 4, 130], BF16, tag="pp4")
                        nc.scalar.activation(
                            pp4[:, :bk, 1:2],
                            sc_ps[:, :bk].rearrange("p (b o) -> p b o", o=1),
                            mybir.ActivationFunctionType.Exp)
                        A8 = sb.tile([128, 4, 128], BF16, tag="A8")
                        nc.vector.tensor_tensor(
                            out=A8[:, :bk, :],
                            in0=dcol[:, b * TPB + t0:b * TPB + t0 + bk]
                                .to_broadcast([128, bk, 128]),
                            in1=iota[:, :bk * 128].rearrange(
                                "p (b e) -> p b e", e=128),
                            op=mybir.AluOpType.is_equal)
                        nc.vector.tensor_tensor(
                            out=pp4[:, :bk, 2:130],
                            in0=u_bin[:, t0:t0 + bk, :],
                            in1=pp4[:, :bk, 1:2].to_broadcast([128, bk, 128]),
                            op=mybir.AluOpType.mult)
                        for j in range(bk):
                            t = t0 + j
                            nc.tensor.matmul(
                                da_ps[:], lhsT=A8[:, j, :],
                                rhs=pp4[:, j, 1:130],
                                start=(t == 0), stop=(t == TPB - 1))
                        t0 += bk

                    # ---- epilogue ----
                    rec = sb.tile([128, 1], F32, tag="rec")
                    nc.vector.reciprocal(rec[:], da_ps[:, 0:1])
                    hp = sb.tile([128, 128], F32, tag="hp")
                    nc.vector.tensor_scalar_mul(hp[:], da_ps[:, 1:129], rec[:])
                    hb = sb.tile([128, 128], F32, tag="hb")
                    nc.vector.tensor_add(
                        hb[:], hp[:], be[:, l * 128:(l + 1) * 128])
                    if l < L - 1:
                        hn = sb.tile([128, 128], F32, tag="hn")
                        nc.scalar.activation(
                            hn[:], hb[:], mybir.ActivationFunctionType.Relu)
                        nc.sync.dma_start(
                            h_cur[b * 128:(b + 1) * 128, :], hn[:])
                    else:
                        nc.scalar.activation(
                            zbuf[:, b * 128:(b + 1) * 128], hb[:],
                            mybir.ActivationFunctionType.Relu)

            # ======== deferred log_softmax over the whole shard ========
            # batched by 4 bins; Ln clusters here so the activation table
            # set switches once instead of twice per bin
            for q0 in range(0, NBPC, 4):
                qn = min(4, NBPC - q0)
                zq = zbuf[:, q0 * 128:(q0 + qn) * 128]
                m4 = sb.tile([128, 4], F32, tag="m4")
                nc.vector.tensor_reduce(
                    m4[:, :qn].rearrange("p (b o) -> p b o", o=1),
                    zq.rearrange("p (b e) -> p b e", e=128),
                    axis=mybir.AxisListType.X, op=mybir.AluOpType.max)
                mn4 = sb.tile([128, 4], F32, tag="mn4")
                nc.scalar.mul(mn4[:, :qn], m4[:, :qn], -1.0)
                zc4 = sb.tile([128, 4 * 128], F32, tag="zc4")
                for j in range(qn):
                    nc.scalar.activation(
                        zc4[:, j * 128:(j + 1) * 128],
                        zbuf[:, (q0 + j) * 128:(q0 + j + 1) * 128],
                        mybir.ActivationFunctionType.Identity,
                        bias=mn4[:, j:j + 1], scale=1.0)
                ez4 = sb.tile([128, 4 * 128], F32, tag="ez4")
                nc.scalar.activation(
                    ez4[:, :qn * 128], zc4[:, :qn * 128],
                    mybir.ActivationFunctionType.Exp)
                s4 = sb.tile([128, 4], F32, tag="s4")
                nc.vector.tensor_reduce(
                    s4[:, :qn].rearrange("p (b o) -> p b o", o=1),
                    ez4[:, :qn * 128].rearrange("p (b e) -> p b e", e=128),
                    axis=mybir.AxisListType.X, op=mybir.AluOpType.add)
                ls4 = sb.tile([128, 4], F32, tag="ls4")
                nc.scalar.activation(
                    ls4[:, :qn], s4[:, :qn],
                    mybir.ActivationFunctionType.Ln)
                lsn4 = sb.tile([128, 4], F32, tag="lsn4")
                nc.scalar.mul(lsn4[:, :qn], ls4[:, :qn], -1.0)
                o4 = sb.tile([128, 4 * 128], F32, tag="o4")
                for j in range(qn):
                    nc.scalar.activation(
                        o4[:, j * 128:(j + 1) * 128],
                        zc4[:, j * 128:(j + 1) * 128],
                        mybir.ActivationFunctionType.Identity,
                        bias=lsn4[:, j:j + 1], scale=1.0)
                for j in range(qn):
                    nc.sync.dma_start(
                        out_t[(q0 + j) * 128:(q0 + j + 1) * 128, :],
                        o4[:, j * 128:(j + 1) * 128])

    nc.compile()
    return nc


# ----------------------------------------------------------------------------
# Entry point
# ----------------------------------------------------------------------------

def kernel(x, edge_index, Wl, bl, Wr, br, att, bias):
    n_cores = 8
    _sim = False
    x = np.asarray(x)
    edge_index = np.asarray(edge_index)
    L = np.asarray(Wl).shape[0]
    p = host_prep(x, edge_index, n_cores)
    w = host_weights(np.asarray(Wl), np.asarray(bl), np.asarray(Wr),
                     np.asarray(br), np.asarray(att), np.asarray(bias), L)
    nc = build_nc(p, L=L)

    in_maps = []
    for c in range(n_cores):
        m = {
            "x_shard": p.core_x[c],
            "gidx": p.core_gidx[c],
            "dcol": p.core_dcol[c],
            "umat": p.core_U[c],
            "wl": w["wl"], "wr": w["wr"], "att": w["att"],
            "att2": w["att2"], "lrb": w["lrb"], "lrbn": w["lrbn"],
            "be": w["be"],
            "idf": w["idf"], "idb": w["idb"], "iota": w["iota"],
        }
        in_maps.append(m)

    if _sim:
        from concourse.bass_interp import CoreSim, MultiCoreSim
        if n_cores == 1:
            sims = [CoreSim(nc)]
            for k, v in in_maps[0].items():
                sims[0].tensor(k)[:] = v
            sims[0].simulate()
            shard_outs = [sims[0].tensor("out").copy()]
        else:
            msim = MultiCoreSim(nc, num_cores=n_cores, trace=False)
            for c, core in sorted(msim.cores.items()):
                for k, v in in_maps[c].items():
                    core.tensor(k)[:] = v
            msim.simulate()
            shard_outs = [msim.cores[c].tensor("out").copy()
                          for c in range(n_cores)]
        res = None
    else:
        res = run_bass_kernel_spmd(nc, in_maps, core_ids=list(range(n_cores)))
        shard_outs = [res.results[c]["out"] for c in range(n_cores)]

    # un-permute
    out = np.zeros((p.N, D), dtype=np.float32)
    for c in range(n_cores):
        nodes = p.bin_nodes[c * p.NBPC:(c + 1) * p.NBPC].reshape(-1)
        valid = nodes >= 0
        out[nodes[valid]] = shard_outs[c][valid]
    kernel.last_results = res
    return out



# revision 15
# speedup vs baseline: 1.4231x; 1.0481x over previous
"""GATv2 (3-layer, H=1, D=128) on 8 Trainium2 NeuronCores via Bass/Tile.

Self-contained: host preprocessing (bin packing, edge tiling, int16 gather
indices), bass program builder, and the `kernel(**inputs)` entry point.

Strategy (dst-node sharding, uniform SPMD program):
  - Permute nodes into bins of 128 dst rows, 2-D balanced by (low,high)
    in-degree so every bin has exactly TL low + TH high edge tiles of 128.
  - Per layer: each core computes xl/xr for its shard (bf16), AllGather of
    xl -> xl_full in every core's HBM.
  - Per bin: dma_gather source features u[e,d] (bf16, 256B rows); per edge
    tile: PE transpose u -> wT psum, += xr^T-onehot matmul (U streamed from
    HBM); ACT LeakyRelu(+bl+br bias); score = lreluT^T @ att (PE);
    exp (ACT); denom/agg via one-hot matmuls accumulated in PSUM; epilogue
    recip + bias + relu (+ log_softmax on the last layer).
"""
import numpy as np
import ml_dtypes

import concourse.bacc as bacc
import concourse.mybir as mybir
import concourse.tile as tile
from concourse.bass_utils import run_bass_kernel_spmd

F32 = mybir.dt.float32
BF16 = mybir.dt.bfloat16
I16 = mybir.dt.int16
SLOPE = 0.2
D = 128
LOWCAP = 32768
DUMMY_DCOL = 200.0  # never matches iota 0..127 -> zero one-hot column


# ----------------------------------------------------------------------------
# Host preprocessing
# ----------------------------------------------------------------------------

def _pack_bins(n_bins, node_ids, w_low, w_high, cap=128,
               cap_low=np.inf, cap_high=np.inf):
    order = np.argsort(-(w_low + w_high), kind="stable")
    bins = [[] for _ in range(n_bins)]
    sums = np.zeros((n_bins, 2))
    counts = np.zeros(n_bins, dtype=np.int64)
    tgt_l = max(w_low.sum() / n_bins, 1.0)
    tgt_h = max(w_high.sum() / n_bins, 1.0)
    for idx in order:
        load = (sums[:, 0] + w_low[idx]) / tgt_l + (sums[:, 1] + w_high[idx]) / tgt_h
        load = np.where(counts >= cap, np.inf, load)
        load = np.where(sums[:, 0] + w_low[idx] > cap_low, np.inf, load)
        load = np.where(sums[:, 1] + w_high[idx] > cap_high, np.inf, load)
        if not np.isfinite(load).any():
            return None, None
        b = int(np.argmin(load))
        bins[b].append(node_ids[idx])
        sums[b, 0] += w_low[idx]
        sums[b, 1] += w_high[idx]
        counts[b] += 1
    return bins, sums


def _pack_bins_capped(n_bins, node_ids, w_low, w_high, cap=128):
    """Try progressively looser (cap_low, cap_high) slot caps so the max
    bin needs fewer 128-edge tiles; fall back to uncapped."""
    for cl, ch in ((1530, 764), (1530, 890), (1658, 890),
                   (np.inf, np.inf)):
        bins, sums = _pack_bins(n_bins, node_ids, w_low, w_high, cap=cap,
                                cap_low=cl, cap_high=ch)
        if bins is not None:
            return bins, sums
    raise AssertionError("unreachable")


def _chunks(n, cap=12):
    out = []
    while n > 0:
        c = min(n, cap)
        out.append(c)
        n -= c
    return out


class Plan:
    pass


def host_prep(x, edge_index, n_cores):
    """Returns Plan with per-core input tensors + structural constants."""
    N = x.shape[0]
    E = edge_index.shape[1]
    src = np.concatenate([np.asarray(edge_index[0], dtype=np.int64),
                          np.arange(N, dtype=np.int64)])
    dst = np.concatenate([np.asarray(edge_index[1], dtype=np.int64),
                          np.arange(N, dtype=np.int64)])

    NB = int(np.ceil(N / 128.0))
    NB = int(np.ceil(NB / n_cores)) * n_cores
    NPAD = NB * 128
    assert NPAD <= 65536
    lowcap = min(LOWCAP, NPAD)  # small configs: everything "low"
    hibase = NPAD - lowcap

    deg = np.bincount(dst, minlength=N).astype(np.float64)
    if NPAD > lowcap:
        # pass 1: balance total degree; defines the low/high node split
        bins1, _ = _pack_bins(NB, np.arange(N), deg, np.zeros(N))
        n_low_bins = lowcap // 128
        low_nodes = np.concatenate(
            [np.asarray(bins1[i], dtype=np.int64) for i in range(n_low_bins)]
        ) if n_low_bins else np.zeros(0, dtype=np.int64)
        is_low = np.zeros(N, dtype=bool)
        is_low[low_nodes] = True
        deg_low = np.bincount(dst[is_low[src]], minlength=N).astype(np.float64)
        deg_high = deg - deg_low
        ln = np.flatnonzero(is_low)
        hn = np.flatnonzero(~is_low)
        bins_l, _ = _pack_bins_capped(n_low_bins, ln, deg_low[ln],
                                      deg_high[ln])
        bins_h, _ = _pack_bins_capped(NB - n_low_bins, hn, deg_low[hn],
                                      deg_high[hn])
        bins = bins_l + bins_h
    else:
        bins, _ = _pack_bins(NB, np.arange(N), deg, np.zeros(N))
        is_low = np.ones(N, dtype=bool)

    # permutation: pos[old] = new index; bins padded with virtual rows
    pos = np.full(N, -1, dtype=np.int64)
    bin_nodes = np.full((NB, 128), -1, dtype=np.int64)
    for b, nodes in enumerate(bins):
        nodes = np.asarray(nodes, dtype=np.int64)
        bin_nodes[b, :len(nodes)] = nodes
        pos[nodes] = b * 128 + np.arange(len(nodes))

    # per-edge: bin, dst_local, low/high, gather idx
    e_pos_dst = pos[dst]
    e_bin = e_pos_dst // 128
    e_row = e_pos_dst % 128
    e_pos_src = pos[src]
    e_is_low = e_pos_src < lowcap

    # group edges per (bin, range)
    # sort by (bin, is_low desc) so lows first
    order = np.lexsort((~e_is_low, e_bin))
    sb, srow, sidx, slow = e_bin[order], e_row[order], e_pos_src[order], e_is_low[order]

    nl = np.zeros(NB, dtype=np.int64)
    nh = np.zeros(NB, dtype=np.int64)
    np.add.at(nl, e_bin[e_is_low], 1)
    np.add.at(nh, e_bin[~e_is_low], 1)
    # virtual rows get a self-edge in the low group
    nvirt = (bin_nodes < 0).sum(axis=1)
    TL = int(np.ceil((nl + nvirt).max() / 128.0))
    TH = int(np.ceil(nh.max() / 128.0)) if NPAD > lowcap else 0
    TPB = TL + TH
    low_calls = _chunks(TL)
    high_calls = _chunks(TH)
    calls_per_bin = [(c, 0) for c in low_calls] + [(c, 1) for c in high_calls]

    # slot arrays per bin: idx (int16), dcol (float), U one-hot
    slot_idx = np.zeros((NB, TPB * 128), dtype=np.int64)
    slot_dcol = np.full((NB, TPB * 128), DUMMY_DCOL, dtype=np.float64)
    # fill real edges
    # low edges occupy slots [0, nl), high edges [TL*128, TL*128+nh)
    starts = np.searchsorted(sb, np.arange(NB))
    ends = np.searchsorted(sb, np.arange(NB), side="right")
    for b in range(NB):
        s, e = starts[b], ends[b]
        lo_cnt = int(np.count_nonzero(slow[s:e]))
        hi_cnt = (e - s) - lo_cnt
        assert lo_cnt <= TL * 128 - nvirt[b] and hi_cnt <= TH * 128
        slot_idx[b, :lo_cnt] = sidx[s:s + lo_cnt]
        slot_dcol[b, :lo_cnt] = srow[s:s + lo_cnt]
        # virtual self-edges right after real low edges
        nv = int(nvirt[b])
        if nv:
            vrows = np.flatnonzero(bin_nodes[b] < 0)
            slot_idx[b, lo_cnt:lo_cnt + nv] = 0
            slot_dcol[b, lo_cnt:lo_cnt + nv] = vrows
        if hi_cnt:
            slot_idx[b, TL * 128:TL * 128 + hi_cnt] = sidx[s + lo_cnt:e] - hibase
            slot_dcol[b, TL * 128:TL * 128 + hi_cnt] = srow[s + lo_cnt:e]
    assert slot_idx.min() >= 0 and slot_idx.max() < 32768

    # wrapped int16 gather indices per call, replicated to 8 groups of 16
    def wrap(idx_flat):
        ni = len(idx_flat)
        a = idx_flat.astype(np.int16).reshape(ni // 16, 16).T  # [16, ni/16]
        w = np.tile(a, (8, 1))  # [128, ni/16]
        return w

    core_gidx = []
    core_dcol = []
    core_U = []
    core_x = []
    NBPC = NB // n_cores
    for c in range(n_cores):
        gx = []
        for b in range(c * NBPC, (c + 1) * NBPC):
            off = 0
            for ntile, rng in calls_per_bin:
                lo = (0 if rng == 0 else TL * 128) + (off if rng == 0 else off - TL * 128)
                gx.append(wrap(slot_idx[b, off:off + ntile * 128]))
                off += ntile * 128
            assert off == TPB * 128
        core_gidx.append(np.concatenate(gx, axis=1))
        dc = slot_dcol[c * NBPC:(c + 1) * NBPC].reshape(NBPC * TPB, 128)
        # dcol layout [128 partitions(e), NBPC*TPB]
        core_dcol.append(
            np.ascontiguousarray(dc.T).astype(ml_dtypes.bfloat16))
        # U one-hot [128 i, tiles*128 e]
        dloc = slot_dcol[c * NBPC:(c + 1) * NBPC].reshape(-1)  # [NBPC*TPB*128]
        U = (dloc[None, :] == np.arange(128)[:, None]).astype(ml_dtypes.bfloat16)
        core_U.append(np.ascontiguousarray(U))
        # x shard in permuted order (virtual rows zero)
        xs = np.zeros((NBPC * 128, D), dtype=np.float32)
        nodes = bin_nodes[c * NBPC:(c + 1) * NBPC].reshape(-1)
        valid = nodes >= 0
        xs[valid] = np.asarray(x)[nodes[valid]]
        core_x.append(xs)

    p = Plan()
    p.N, p.E, p.NB, p.NPAD, p.NBPC = N, E, NB, NPAD, NBPC
    p.TL, p.TH, p.TPB = TL, TH, TPB
    p.calls_per_bin = calls_per_bin
    p.lowcap, p.hibase = lowcap, hibase
    p.n_cores = n_cores
    p.core_gidx, p.core_dcol, p.core_U, p.core_x = (
        core_gidx, core_dcol, core_U, core_x)
    p.bin_nodes = bin_nodes
    p.pos = pos
    p.idx_cols = core_gidx[0].shape[1]
    return p


def host_weights(Wl, bl, Wr, br, att, bias, L):
    """Shared (replicated) weight tensors."""
    w = {}
    w["wl"] = np.ascontiguousarray(
        np.concatenate([np.asarray(Wl[l], np.float32) for l in range(L)], axis=1)
    ).astype(ml_dtypes.bfloat16)  # [128, L*128]
    w["wr"] = np.ascontiguousarray(
        np.concatenate([np.asarray(Wr[l], np.float32) for l in range(L)], axis=1)
    ).astype(ml_dtypes.bfloat16)
    w["att"] = np.ascontiguousarray(
        np.stack([np.asarray(att[l, 0], np.float32) for l in range(L)], axis=1)
    ).astype(ml_dtypes.bfloat16)  # [128, L]
    w["att2"] = np.ascontiguousarray(
        np.stack([np.asarray(-0.2 * att[l, 0], np.float32) for l in range(L)],
                 axis=1)).astype(ml_dtypes.bfloat16)  # [128, L]
    # lrelu bias = bl + br per layer, [128, L] f32
    w["lrb"] = np.ascontiguousarray(
        np.stack([np.asarray(bl[l] + br[l], np.float32) for l in range(L)], axis=1))
    w["lrbn"] = np.ascontiguousarray(-w["lrb"])
    # epilogue bias = bl + bias per layer, replicated [128, L*128] f32
    be = np.concatenate(
        [np.tile((np.asarray(bl[l] + bias[l], np.float32))[None, :], (128, 1))
         for l in range(L)], axis=1)
    w["be"] = np.ascontiguousarray(be)
    ident = np.eye(128, dtype=np.float32)
    w["idf"] = ident
    w["idb"] = ident.astype(ml_dtypes.bfloat16)
    w["iota"] = np.tile(np.arange(128, dtype=np.float32)[None, :], (128, 32)
                        ).astype(ml_dtypes.bfloat16)  # [128, 32*128]
    return w


# ----------------------------------------------------------------------------
# Device program
# ----------------------------------------------------------------------------

def build_nc(p, L=3):
    NBPC, TL, TH, TPB = p.NBPC, p.TL, p.TH, p.TPB
    NSH = NBPC * 128          # shard rows
    n_cores = p.n_cores
    NPAD = p.NPAD

    nc = bacc.Bacc("TRN2", target_bir_lowering=False, debug=False,
                   num_devices=n_cores, num_swdge_queues=4)

    # --- I/O ---
    x_in = nc.dram_tensor("x_shard", [NSH, D], F32, kind="ExternalInput")
    gidx_in = nc.dram_tensor("gidx", [128, p.idx_cols], I16, kind="ExternalInput")
    dcol_in = nc.dram_tensor("dcol", [128, NBPC * TPB], BF16, kind="ExternalInput")
    U_in = nc.dram_tensor("umat", [128, NBPC * TPB * 128], BF16,
                          kind="ExternalInput")
    wl_in = nc.dram_tensor("wl", [128, L * 128], BF16, kind="ExternalInput")
    wr_in = nc.dram_tensor("wr", [128, L * 128], BF16, kind="ExternalInput")
    att_in = nc.dram_tensor("att", [128, L], BF16, kind="ExternalInput")
    att2_in = nc.dram_tensor("att2", [128, L], BF16, kind="ExternalInput")
    lrb_in = nc.dram_tensor("lrb", [128, L], F32, kind="ExternalInput")
    lrbn_in = nc.dram_tensor("lrbn", [128, L], F32, kind="ExternalInput")
    be_in = nc.dram_tensor("be", [128, L * 128], F32, kind="ExternalInput")
    idf_in = nc.dram_tensor("idf", [128, 128], F32, kind="ExternalInput")
    idb_in = nc.dram_tensor("idb", [128, 128], BF16, kind="ExternalInput")
    iota_in = nc.dram_tensor("iota", [128, 32 * 128], BF16,
                             kind="ExternalInput")
    out_t = nc.dram_tensor("out", [NSH, D], F32, kind="ExternalOutput")

    with tile.TileContext(nc) as tc:
        with (
            tc.tile_pool(name="const", bufs=1) as constp,
            tc.tile_pool(name="sb", bufs=2) as sb,
            tc.tile_pool(name="sb3", bufs=6) as sb3,
            tc.tile_pool(name="psum", bufs=2, space="PSUM") as ps,
            tc.tile_pool(name="dram", bufs=1, space="DRAM") as dram,
        ):
            # ---- constants / resident tensors ----
            gidx = constp.tile([128, p.idx_cols], I16)
            nc.sync.dma_start(gidx[:], gidx_in[:])
            dcol = constp.tile([128, NBPC * TPB], BF16)
            nc.sync.dma_start(dcol[:], dcol_in[:])
            wl = constp.tile([128, L * 128], BF16)
            nc.sync.dma_start(wl[:], wl_in[:])
            wr = constp.tile([128, L * 128], BF16)
            nc.sync.dma_start(wr[:], wr_in[:])
            att = constp.tile([128, L], BF16)
            nc.sync.dma_start(att[:], att_in[:])
            att2 = constp.tile([128, L], BF16)
            nc.sync.dma_start(att2[:], att2_in[:])
            lrb = constp.tile([128, L], F32)
            nc.sync.dma_start(lrb[:], lrb_in[:])
            lrbn = constp.tile([128, L], F32)
            nc.sync.dma_start(lrbn[:], lrbn_in[:])
            be = constp.tile([128, L * 128], F32)
            nc.sync.dma_start(be[:], be_in[:])
            idf = constp.tile([128, 128], F32)
            nc.sync.dma_start(idf[:], idf_in[:])
            idb = constp.tile([128, 128], BF16)
            nc.sync.dma_start(idb[:], idb_in[:])
            iota = constp.tile([128, 32 * 128], BF16)
            nc.sync.dma_start(iota[:], iota_in[:])
            # xr kept SBUF-resident per layer
            xr_res = constp.tile([128, NBPC * 128], BF16)
            # relu(h) of the last layer, held for the deferred log_softmax
            zbuf = constp.tile([128, NBPC * 128], F32)

            # ---- DRAM scratch ----
            h_cur = dram.tile([NSH, D], F32)       # current layer input
            xl_shard = dram.tile([NSH, D], BF16)
            xl_full = dram.tile([NPAD, D], BF16)
            nc.sync.dma_start(h_cur[:], x_in[:])

            for l in range(L):
                # ======== Phase A: xl/xr shard + AllGather ========
                for t in range(NBPC):
                    h_t = sb.tile([128, 128], F32, tag="ht")
                    nc.sync.dma_start(h_t[:], h_cur[t * 128:(t + 1) * 128, :])
                    hT_ps = ps.tile([128, 128], F32, tag="wT")
                    nc.tensor.matmul(hT_ps[:], lhsT=h_t[:], rhs=idf[:],
                                     start=True, stop=True)
                    hT = sb.tile([128, 128], BF16, tag="hT")
                    nc.vector.tensor_copy(hT[:], hT_ps[:])
                    xl_ps = ps.tile([128, 128], F32, tag="ag")
                    nc.tensor.matmul(xl_ps[:], lhsT=hT[:],
                                     rhs=wl[:, l * 128:(l + 1) * 128],
                                     start=True, stop=True)
                    xl_t = sb.tile([128, 128], BF16, tag="xlt")
                    nc.vector.tensor_copy(xl_t[:], xl_ps[:])
                    nc.sync.dma_start(xl_shard[t * 128:(t + 1) * 128, :], xl_t[:])
                    xr_ps = ps.tile([128, 128], F32, tag="sc")
                    nc.tensor.matmul(xr_ps[:], lhsT=hT[:],
                                     rhs=wr[:, l * 128:(l + 1) * 128],
                                     start=True, stop=True)
                    nc.vector.tensor_copy(
                        xr_res[:, t * 128:(t + 1) * 128], xr_ps[:])

                nc.gpsimd.collective_compute(
                    "AllGather",
                    mybir.AluOpType.bypass,
                    replica_groups=[list(range(n_cores))],
                    ins=[xl_shard.opt()],
                    outs=[xl_full.opt()],
                )

                # ======== Phase B: per-bin edge processing ========
                idx_off = 0
                for b in range(NBPC):
                    u_bin = sb3.tile([128, TPB, 128], BF16, tag="u")
                    # gather calls
                    tile_off = 0
                    for ci, (ntile, rng) in enumerate(p.calls_per_bin):
                        ni = ntile * 128
                        base = 0 if rng == 0 else p.hibase
                        src_view = xl_full[base:base + p.lowcap, :]
                        nc.gpsimd.dma_gather(
                            out_ap=u_bin[:, tile_off:tile_off + ntile, :],
                            in_ap=src_view,
                            idxs_ap=gidx[:, idx_off:idx_off + ni // 16],
                            num_idxs=ni,
                            num_idxs_reg=ni,
                            elem_size=D,
                            queue_num=(b * len(p.calls_per_bin) + ci) % 4,
                            single_packet=False,
                        )
                        idx_off += ni // 16
                        tile_off += ntile
                    # U for this bin
                    U_bin = sb3.tile([128, TPB * 128], BF16, tag="U")
                    nc.sync.dma_start(
                        U_bin[:],
                        U_in[:, (b * TPB) * 128:((b + 1) * TPB) * 128])

                    xr_tile = xr_res[:, b * 128:(b + 1) * 128]
                    da_ps = ps.tile([128, 129], F32, tag="da")

                    # batches of up to 4 tiles
                    bk_sizes = _chunks(TPB, 4)
                    sc_ps = ps.tile([128, TPB], F32, tag="sc")
                    t0 = 0
                    for bk in bk_sizes:
                        wT_ps = ps.tile([128, 4 * 128], F32, tag="wT")
                        # one wide xr@U matmul seeds the chunk (1 LDW), then
                        # per-tile transposes accumulate on top
                        nc.tensor.matmul(
                            wT_ps[:, :bk * 128],
                            lhsT=xr_tile,
                            rhs=U_bin[:, t0 * 128:(t0 + bk) * 128],
                            start=True, stop=False)
                        for j in range(bk):
                            t = t0 + j
                            nc.tensor.matmul(
                                wT_ps[:, j * 128:(j + 1) * 128],
                                lhsT=u_bin[:, t, :], rhs=idb[:],
                                start=False, stop=True)
                        lre = sb.tile([128, 4 * 128], BF16, tag="lre")
                        nc.scalar.activation(
                            lre[:, :bk * 128], wT_ps[:, :bk * 128],
                            mybir.ActivationFunctionType.Relu,
                            bias=lrb[:, l:l + 1], scale=1.0)
                        lrn = sb.tile([128, 4 * 128], BF16, tag="lrn")
                        nc.scalar.activation(
                            lrn[:, :bk * 128], wT_ps[:, :bk * 128],
                            mybir.ActivationFunctionType.Relu,
                            bias=lrbn[:, l:l + 1], scale=-1.0)
                        for j in range(bk):
                            t = t0 + j
                            nc.tensor.matmul(
                                sc_ps[:, t:t + 1],
                                lhsT=lre[:, j * 128:(j + 1) * 128],
                                rhs=att[:, l:l + 1], start=True, stop=False)
                            nc.tensor.matmul(
                                sc_ps[:, t:t + 1],
                                lhsT=lrn[:, j * 128:(j + 1) * 128],
                                rhs=att2[:, l:l + 1], start=False, stop=True)
                        t0 += bk

                    # one exp + one pu-mult + one A8 build for the whole bin;
                    # pp layout per tile: [pad, p8, pu(128)] so denom+agg is
                    # one matmul streaming cols 1:130
                    pp_b = sb.tile([128, TPB, 130], BF16, tag="pp4")
                    nc.scalar.activation(
                        pp_b[:, :, 1:2],
                        sc_ps[:].rearrange("p (b o) -> p b o", o=1),
                        mybir.ActivationFunctionType.Exp)
                    A8 = sb.tile([128, TPB, 128], BF16, tag="A8")
                    nc.vector.tensor_tensor(
                        out=A8[:],
                        in0=dcol[:, b * TPB:(b + 1) * TPB]
                            .to_broadcast([128, TPB, 128]),
                        in1=iota[:, :TPB * 128].rearrange(
                            "p (b e) -> p b e", e=128),
                        op=mybir.AluOpType.is_equal)
                    nc.vector.tensor_tensor(
                        out=pp_b[:, :, 2:130],
                        in0=u_bin[:],
                        in1=pp_b[:, :, 1:2].to_broadcast([128, TPB, 128]),
                        op=mybir.AluOpType.mult)
                    for t in range(TPB):
                        nc.tensor.matmul(
                            da_ps[:], lhsT=A8[:, t, :],
                            rhs=pp_b[:, t, 1:130],
                            start=(t == 0), stop=(t == TPB - 1))

                    # ---- epilogue ----
                    rec = sb.tile([128, 1], F32, tag="rec")
                    nc.vector.reciprocal(rec[:], da_ps[:, 0:1])
                    hp = sb.tile([128, 128], F32, tag="hp")
                    nc.vector.tensor_scalar_mul(hp[:], da_ps[:, 1:129], rec[:])
                    hb = sb.tile([128, 128], F32, tag="hb")
                    nc.vector.tensor_add(
                        hb[:], hp[:], be[:, l * 128:(l + 1) * 128])
                    if l < L - 1:
                        hn = sb.tile([128, 128], F32, tag="hn")
                        nc.scalar.activation(
                            hn[:], hb[:], mybir.ActivationFunctionType.Relu)
                        nc.sync.dma_start(
                            h_cur[b * 128:(b + 1) * 128, :], hn[:])
                    else:
                        nc.scalar.activation(
                            zbuf[:, b * 128:(b + 1) * 128], hb[:],
                            mybir.ActivationFunctionType.Relu)

            # ======== deferred log_softmax over the whole shard ========
            # batched by 4 bins; Ln clusters here so the activation table
            # set switches once instead of twice per bin
            for q0 in range(0, NBPC, 4):
                qn = min(4, NBPC - q0)
                zq = zbuf[:, q0 * 128:(q0 + qn) * 128]
                m4 = sb.tile([128, 4], F32, tag="m4")
                nc.vector.tensor_reduce(
                    m4[:, :qn].rearrange("p (b o) -> p b o", o=1),
                    zq.rearrange("p (b e) -> p b e", e=128),
                    axis=mybir.AxisListType.X, op=mybir.AluOpType.max)
                zc4 = sb.tile([128, 4 * 128], F32, tag="zc4")
                nc.vector.tensor_tensor(
                    out=zc4[:, :qn * 128].rearrange(
                        "p (b e) -> p b e", e=128),
                    in0=zq.rearrange("p (b e) -> p b e", e=128),
                    in1=m4[:, :qn].rearrange("p (b o) -> p b o", o=1)
                        .to_broadcast([128, qn, 128]),
                    op=mybir.AluOpType.subtract)
                ez4 = sb.tile([128, 4 * 128], F32, tag="ez4")
                nc.scalar.activation(
                    ez4[:, :qn * 128], zc4[:, :qn * 128],
                    mybir.ActivationFunctionType.Exp)
                s4 = sb.tile([128, 4], F32, tag="s4")
                nc.vector.tensor_reduce(
                    s4[:, :qn].rearrange("p (b o) -> p b o", o=1),
                    ez4[:, :qn * 128].rearrange("p (b e) -> p b e", e=128),
                    axis=mybir.AxisListType.X, op=mybir.AluOpType.add)
                ls4 = sb.tile([128, 4], F32, tag="ls4")
                nc.scalar.activation(
                    ls4[:, :qn], s4[:, :qn],
                    mybir.ActivationFunctionType.Ln)
                o4 = sb.tile([128, 4 * 128], F32, tag="o4")
                nc.vector.tensor_tensor(
                    out=o4[:, :qn * 128].rearrange(
                        "p (b e) -> p b e", e=128),
                    in0=zc4[:, :qn * 128].rearrange(
                        "p (b e) -> p b e", e=128),
                    in1=ls4[:, :qn].rearrange("p (b o) -> p b o", o=1)
                        .to_broadcast([128, qn, 128]),
                    op=mybir.AluOpType.subtract)
                for j in range(qn):
                    nc.sync.dma_start(
                        out_t[(q0 + j) * 128:(q0 + j + 1) * 128, :],
                        o4[:, j * 128:(j + 1) * 128])

    nc.compile()
    return nc


# ----------------------------------------------------------------------------
# Entry point
# ----------------------------------------------------------------------------

def kernel(x, edge_index, Wl, bl, Wr, br, att, bias):
    n_cores = 8
    _sim = False
    x = np.asarray(x)
    edge_index = np.asarray(edge_index)
    L = np.asarray(Wl).shape[0]
    p = host_prep(x, edge_index, n_cores)
    w = host_weights(np.asarray(Wl), np.asarray(bl), np.asarray(Wr),
                     np.asarray(br), np.asarray(att), np.asarray(bias), L)
    nc = build_nc(p, L=L)

    in_maps = []
    for c in range(n_cores):
        m = {
            "x_shard": p.core_x[c],
            "gidx": p.core_gidx[c],
            "dcol": p.core_dcol[c],
            "umat": p.core_U[c],
            "wl": w["wl"], "wr": w["wr"], "att": w["att"],
            "att2": w["att2"], "lrb": w["lrb"], "lrbn": w["lrbn"],
            "be": w["be"],
            "idf": w["idf"], "idb": w["idb"], "iota": w["iota"],
        }
        in_maps.append(m)

    if _sim:
        from concourse.bass_interp import CoreSim, MultiCoreSim
        if n_cores == 1:
            sims = [CoreSim(nc)]
            for k, v in in_maps[0].items():
                sims[0].tensor(k)[:] = v
            sims[0].simulate()
            shard_outs = [sims[0].tensor("out").copy()]
        else:
            msim = MultiCoreSim(nc, num_cores=n_cores, trace=False)
            for c, core in sorted(msim.cores.items()):
                for k, v in in_maps[c].items():
                    core.tensor(k)[:] = v
            msim.simulate()
            shard_outs = [msim.cores[c].tensor("out").copy()
                          for c in range(n_cores)]
        res = None
    else:
        res = run_bass_kernel_spmd(nc, in_maps, core_ids=list(range(n_cores)))
        shard_outs = [res.results[c]["out"] for c in range(n_cores)]

    # un-permute
    out = np.zeros((p.N, D), dtype=np.float32)
    for c in range(n_cores):
        nodes = p.bin_nodes[c * p.NBPC:(c + 1) * p.NBPC].reshape(-1)
        valid = nodes >= 0
        out[nodes[valid]] = shard_outs[c][valid]
    kernel.last_results = res
    return out

